# revision 1
# baseline (speedup 1.0000x reference)
"""Trainium2 Bass kernel for a dense GQA transformer layer (pre-norm, SwiGLU MLP).

Full shapes: B=2, S=2048, H=2048, NH=16, NKV=8, HD=128, FF=5632, fp32 I/O.

Sharding across 8 NeuronCores (one SPMD program):
  core = (b, r) with b = core//4 (data-parallel over batch),
  r = core%4 (sequence-parallel, row-interleaved: core owns rows r::4 of
  batch b). Row interleaving makes the causal-attention work identical on
  every core, which a single SPMD program requires.
  K/V are computed for owned rows only and AllGather'ed (groups of 4).
  Everything else (QKV/O projections, softmax, MLP) is token-parallel with
  full weights per core. Host reassembles the row-interleaved outputs.

Precision: bf16 matmuls with fp32 PSUM accumulation; softmax, norms and
residuals in fp32. RMSNorm weights are folded into the following projection
weights host-side; all weights are pre-transposed host-side to [in, out].
"""

import sys

if "/opt/trn_rl_repo" not in sys.path:
    sys.path.insert(0, "/opt/trn_rl_repo")

import math
import os
import numpy as np
import ml_dtypes

import concourse.bass as bass
import concourse.bacc as bacc
import concourse.tile as tile
import concourse.mybir as mybir
from concourse.bass_utils import run_bass_kernel_spmd
from concourse.masks import make_identity

F32 = mybir.dt.float32
BF16 = mybir.dt.bfloat16
AFT = mybir.ActivationFunctionType
ALU = mybir.AluOpType

# ---- fixed problem dims ----
B, S, H = 2, 2048, 2048
NH, NKV, HD = 16, 8, 128
FF = 5632
EPS = 1e-6
NC = 8          # cores
TPG = 4         # cores per batch group (sequence-parallel ways)
P = 128         # partitions

MASK_CLAMP = -30000.0


def _build_program(S_, FF_, ext, masked, n_mask):
    """Emit the SPMD program.

    S_: sequence length, FF_: mlp width (parameterized for small-scale tests)
    ext: tuple, per q-tile number of 512-col key banks to compute
    masked: dict {(qt, bank): mask_slot_index} for banks needing a mask add
    n_mask: number of [128, 512] mask blocks in the mask input
    """
    TOWN = S_ // TPG              # tokens owned per core
    NT = TOWN // P                # q-tiles per core
    NB = S_ // 512                # key banks (512 cols each)
    HT = H // P                   # 16 H tiles
    FC = FF_ // P                 # FF tiles
    KVH = NKV
    assert len(ext) == NT
    QSCALE = 1.0 / math.sqrt(HD)

    nc = bacc.Bacc("TRN2", target_bir_lowering=False, debug=False,
                   num_devices=NC)

    # ---- I/O ----
    x_in = nc.dram_tensor("x", [NT, P, H], F32, kind="ExternalInput").ap()
    wqT = nc.dram_tensor("wqT", [H, NH * HD], BF16, kind="ExternalInput").ap()
    wkT = nc.dram_tensor("wkT", [H, NKV * HD], BF16, kind="ExternalInput").ap()
    wvT = nc.dram_tensor("wvT", [H, NKV * HD], BF16, kind="ExternalInput").ap()
    woT = nc.dram_tensor("woT", [NH * HD, H], BF16, kind="ExternalInput").ap()
    wgT = nc.dram_tensor("wgT", [H, FF_], BF16, kind="ExternalInput").ap()
    wuT = nc.dram_tensor("wuT", [H, FF_], BF16, kind="ExternalInput").ap()
    wdT = nc.dram_tensor("wdT", [FF_, H], BF16, kind="ExternalInput").ap()
    mask_in = nc.dram_tensor("mask", [max(n_mask, 1), P, 512], F32,
                             kind="ExternalInput").ap()
    y_out = nc.dram_tensor("y", [NT, P, H], F32, kind="ExternalOutput").ap()

    # ---- internal DRAM for the K/V AllGather (split in halves for overlap) ----
    KH = KVH // 2
    k_loc = [nc.dram_tensor(f"k_loc{i}", [KH, HD, NT, P], BF16).ap()
             for i in range(2)]
    v_loc = [nc.dram_tensor(f"v_loc{i}", [NT, P, KH, HD], BF16).ap()
             for i in range(2)]
    k_all = [nc.dram_tensor(f"k_all{i}", [TPG, KH, HD, NT, P], BF16).ap()
             for i in range(2)]
    v_all = [nc.dram_tensor(f"v_all{i}", [TPG, NT, P, KH, HD], BF16).ap()
             for i in range(2)]

    groups = [[g * TPG + i for i in range(TPG)] for g in range(NC // TPG)]

    from contextlib import ExitStack
    with ExitStack() as ctx:
        tc = ctx.enter_context(tile.TileContext(nc))
        pool = lambda name, bufs, **kw: ctx.enter_context(
            tc.tile_pool(name=name, bufs=bufs, **kw))
        singles = pool("ones", 1)
        resid_pool = pool("resid", NT)
        ybuf = pool("ybuf", HT)
        qT_pool = pool("qTp", NT)
        kv_pool = pool("kvbuf", 2)
        scratch_pool = pool("scratch", 1)
        ybf_pool = pool("ybfp", 1)
        pbf_pool = pool("pbf", 4)
        pT_pool = pool("pTp", 2)
        aT_pool = pool("aTp", 1)
        mT_pool = pool("mTp", FC)
        mask_pool = pool("maskp", max(n_mask, 1))
        small_pool = pool("small", 8)
        wrhs_pool = pool("wrhs", 6)
        wlhs_pool = pool("wlhs", 12)
        cpy_pool = pool("cpy", 4)
        ptr_pool = pool("ptr", 2, space="PSUM")
        pmm_pool = pool("pmm", 5, space="PSUM")
        psc_pool = pmm_pool
        pav_pool = pool("pav", 1, space="PSUM")

        ident = singles.tile([P, P], BF16)
        make_identity(nc, ident)
        eps_c = singles.tile([P, 1], F32)
        nc.vector.memset(eps_c, EPS)

        # mask blocks (fp32, resident)
        mask_sb = []
        for mi in range(n_mask):
            mt = mask_pool.tile([P, 512], F32, tag="mask")
            nc.sync.dma_start(out=mt, in_=mask_in[mi])
            mask_sb.append(mt)

        def rmsnorm_to_ybf(xt):
            sq = scratch_pool.tile([P, H], F32, tag="sq")
            ssum = small_pool.tile([P, 1], F32, tag="ss")
            nc.scalar.activation(out=sq, in_=xt, func=AFT.Square,
                                 accum_out=ssum)
            std = small_pool.tile([P, 1], F32, tag="std")
            nc.scalar.activation(out=std, in_=ssum, func=AFT.Sqrt,
                                 scale=1.0 / H, bias=eps_c)
            rstd = small_pool.tile([P, 1], F32, tag="rstd")
            nc.vector.reciprocal(rstd, std)
            ybf = ybf_pool.tile([P, H], BF16, tag="ybf")
            nc.scalar.activation(out=ybf, in_=xt, func=AFT.Copy, scale=rstd)
            return ybf

        def transpose_into(dst_tiles, ybf, qt):
            # ybf [128, H] (tokens x H) -> dst_tiles[ht][:, qt*P:...] (H x tok)
            for ht in range(HT):
                ptr = ptr_pool.tile([P, 2, P], BF16, tag="tr")
                nc.tensor.transpose(ptr[:, 0, :],
                                    ybf[:, ht * P:(ht + 1) * P], ident)
                nc.vector.tensor_copy(dst_tiles[ht][:, qt * P:(qt + 1) * P],
                                      ptr[:, 0, :])

        # Y[ht] : transposed normed activations [128 (H), TOWN] bf16
        Y = [ybuf.tile([P, TOWN], BF16, tag="y", name=f"Y{i}") for i in range(HT)]
        x_tiles = []

        # ---- stage A: load x, rmsnorm1, transpose into Y ----
        for qt in range(NT):
            xt = resid_pool.tile([P, H], F32, tag="x")
            x_tiles.append(xt)
            nc.sync.dma_start(out=xt, in_=x_in[qt])
            ybf = rmsnorm_to_ybf(xt)
            transpose_into(Y, ybf, qt)

        # ---- stage B: K/V for owned tokens, AllGather ----
        for kvp in range(KVH // 2):        # kv head pairs
            pk = [pmm_pool.tile([P, 512], F32, tag="mm", name=f"pk{j}") for j in range(2)]
            for ht in range(HT):
                wl = wlhs_pool.tile([P, 2, P], BF16, tag="wl")
                nc.sync.dma_start(
                    out=wl.rearrange("k a b -> k (a b)"),
                    in_=wkT[ht * P:(ht + 1) * P,
                            kvp * 2 * HD:(kvp + 1) * 2 * HD])
                for j in range(2):
                    nc.tensor.matmul(pk[j][:, :TOWN], lhsT=wl[:, j, :],
                                     rhs=Y[ht], start=(ht == 0),
                                     stop=(ht == HT - 1))
            for j in range(2):
                kvh = kvp * 2 + j
                kc = cpy_pool.tile([P, 512], BF16, tag="kc")
                nc.vector.tensor_copy(kc[:, :TOWN], pk[j][:, :TOWN])
                nc.sync.dma_start(
                    out=k_loc[kvh // KH][kvh % KH].rearrange(
                        "d t i -> d (t i)"),
                    in_=kc[:, :TOWN])

        for t in range(NT):
            for half in range(2):
                pv = pmm_pool.tile([P, 512], F32, tag="mm")
                for ht in range(HT):
                    wr = wrhs_pool.tile([P, 512], BF16, tag="wr")
                    nc.sync.dma_start(
                        out=wr, in_=wvT[ht * P:(ht + 1) * P,
                                        half * 512:(half + 1) * 512])
                    nc.tensor.matmul(pv, lhsT=Y[ht][:, t * P:(t + 1) * P],
                                     rhs=wr, start=(ht == 0),
                                     stop=(ht == HT - 1))
                vc = cpy_pool.tile([P, 512], BF16, tag="kc")
                nc.vector.tensor_copy(vc, pv)
                nc.sync.dma_start(
                    out=v_loc[half][t].rearrange("s k d -> s (k d)"),
                    in_=vc)

        for i in range(2):
            nc.gpsimd.collective_compute(
                "AllGather", ALU.bypass, ins=[k_loc[i].opt()],
                outs=[k_all[i].opt()], replica_groups=groups)
            nc.gpsimd.collective_compute(
                "AllGather", ALU.bypass, ins=[v_loc[i].opt()],
                outs=[v_all[i].opt()], replica_groups=groups)

        # ---- stage C: Q for owned tokens -> qT[qt] [128(d), NH, 128(tq)] ----
        qT = []
        for qt in range(NT):
            qn = scratch_pool.tile([P, NH * HD], BF16, tag="qn")
            for oc in range(NH * HD // 512):
                pq = pmm_pool.tile([P, 512], F32, tag="mm")
                for ht in range(HT):
                    wr = wrhs_pool.tile([P, 512], BF16, tag="wr")
                    nc.sync.dma_start(
                        out=wr, in_=wqT[ht * P:(ht + 1) * P,
                                        oc * 512:(oc + 1) * 512])
                    nc.tensor.matmul(pq, lhsT=Y[ht][:, qt * P:(qt + 1) * P],
                                     rhs=wr, start=(ht == 0),
                                     stop=(ht == HT - 1))
                nc.vector.tensor_copy(qn[:, oc * 512:(oc + 1) * 512], pq)
            qTt = qT_pool.tile([P, NH, P], BF16, tag="qT")
            qT.append(qTt)
            for h in range(NH):
                ptr = ptr_pool.tile([P, 2, P], BF16, tag="tr")
                nc.tensor.transpose(ptr[:, 0, :], qn[:, h * P:(h + 1) * P],
                                    ident)
                nc.vector.tensor_copy(qTt[:, h, :], ptr[:, 0, :])

        # ---- stage D: attention ----
        aT = aT_pool.tile([P, NH, TOWN], BF16, tag="aT")
        for kvh in range(KVH):
            kT_sb = kv_pool.tile([P, NB, TPG, P], BF16, tag="kT")
            v_sb = kv_pool.tile([P, NB, TPG, HD], BF16, tag="vT")
            ka, va = k_all[kvh // KH], v_all[kvh // KH]
            for o in range(TPG):
                nc.sync.dma_start(out=kT_sb[:, :, o, :], in_=ka[o, kvh % KH])
                nc.sync.dma_start(
                    out=v_sb[:, :, o, :],
                    in_=va[o].rearrange("t s k d -> s t k d")[:, :, kvh % KH, :])
            for qt in range(NT):
                nbank = ext[qt]
                p_bf = []
                for h2 in range(2):
                    h = 2 * kvh + h2
                    accs = small_pool.tile([P, NB], F32, tag="accs")
                    pb = pbf_pool.tile([P, NB * 512], BF16, tag="pb")
                    p_bf.append(pb)
                    for bi in range(nbank):
                        ps = psc_pool.tile([P, 512], F32, tag="mm")
                        nc.tensor.matmul(
                            ps, lhsT=qT[qt][:, h, :],
                            rhs=kT_sb[:, bi, :, :].rearrange(
                                "d o i -> d (o i)"),
                            start=True, stop=True)
                        mi = masked.get((qt, bi))
                        if mi is not None:
                            nc.vector.tensor_add(ps, ps, mask_sb[mi])
                        nc.scalar.activation(
                            out=pb[:, bi * 512:(bi + 1) * 512], in_=ps,
                            func=AFT.Exp, accum_out=accs[:, bi:bi + 1])
                    den = small_pool.tile([P, 1], F32, tag="den")
                    nc.vector.tensor_reduce(den, accs[:, :nbank],
                                            mybir.AxisListType.X, ALU.add)
                    rec = small_pool.tile([P, 1], F32, tag="rec")
                    nc.vector.reciprocal(rec, den)
                    nc.scalar.activation(out=pb[:, :nbank * 512],
                                         in_=pb[:, :nbank * 512],
                                         func=AFT.Copy, scale=rec)
                pav = pav_pool.tile([P, 2, P], F32, tag="av")
                for pc in range(nbank * TPG):
                    pT_ps = ptr_pool.tile([P, 2, P], BF16, tag="tr")
                    for h2 in range(2):
                        nc.tensor.transpose(
                            pT_ps[:, h2, :],
                            p_bf[h2][:, pc * P:(pc + 1) * P], ident)
                    pT_sb = pT_pool.tile([P, 2, P], BF16, tag="pT")
                    nc.vector.tensor_copy(pT_sb, pT_ps)
                    nc.tensor.matmul(
                        pav, lhsT=v_sb[:, pc // TPG, pc % TPG, :],
                        rhs=pT_sb.rearrange("s h i -> s (h i)"),
                        start=(pc == 0), stop=(pc == nbank * TPG - 1))
                nc.vector.tensor_copy(
                    aT[:, 2 * kvh:2 * kvh + 2, qt * P:(qt + 1) * P], pav)

        # ---- stage E: O projection, streamed transpose + residual into x ----
        for hcp in range(HT // 2):         # H-column pairs
            po = [pmm_pool.tile([P, 512], F32, tag="mm", name=f"po{j}") for j in range(2)]
            for h in range(NH):
                wl = wlhs_pool.tile([P, 2, P], BF16, tag="wl")
                nc.sync.dma_start(
                    out=wl.rearrange("k a b -> k (a b)"),
                    in_=woT[h * P:(h + 1) * P, hcp * 2 * P:(hcp + 1) * 2 * P])
                for j in range(2):
                    nc.tensor.matmul(po[j][:, :TOWN], lhsT=wl[:, j, :],
                                     rhs=aT[:, h, :], start=(h == 0),
                                     stop=(h == NH - 1))
            for j in range(2):
                hc = hcp * 2 + j
                oc = cpy_pool.tile([P, 512], BF16, tag="kc")
                nc.vector.tensor_copy(oc[:, :TOWN], po[j][:, :TOWN])
                for qt in range(NT):
                    ptr = ptr_pool.tile([P, 2, P], BF16, tag="tr")
                    nc.tensor.transpose(ptr[:, 0, :],
                                        oc[:, qt * P:(qt + 1) * P], ident)
                    xb = x_tiles[qt][:, hc * P:(hc + 1) * P]
                    nc.vector.tensor_add(xb, xb, ptr[:, 0, :])

        # ---- rmsnorm2 -> Y2 ----
        Y2 = [ybuf.tile([P, TOWN], BF16, tag="y", name=f"Y2_{i}") for i in range(HT)]
        for qt in range(NT):
            ybf = rmsnorm_to_ybf(x_tiles[qt])
            transpose_into(Y2, ybf, qt)

        # ---- stage F: MLP ----
        mT = []
        for fcp in range(FC // 2):         # FF-tile pairs
            pg = [pmm_pool.tile([P, 512], F32, tag="mm", name=f"pg{j}") for j in range(2)]
            for ht in range(HT):
                wl = wlhs_pool.tile([P, 2, P], BF16, tag="wl")
                nc.sync.dma_start(
                    out=wl.rearrange("k a b -> k (a b)"),
                    in_=wgT[ht * P:(ht + 1) * P, fcp * 2 * P:(fcp + 1) * 2 * P])
                for j in range(2):
                    nc.tensor.matmul(pg[j][:, :TOWN], lhsT=wl[:, j, :],
                                     rhs=Y2[ht], start=(ht == 0),
                                     stop=(ht == HT - 1))
            pu = [psc_pool.tile([P, 512], F32, tag="mm", name=f"pu{j}") for j in range(2)]
            for ht in range(HT):
                wl = wlhs_pool.tile([P, 2, P], BF16, tag="wl")
                nc.sync.dma_start(
                    out=wl.rearrange("k a b -> k (a b)"),
                    in_=wuT[ht * P:(ht + 1) * P, fcp * 2 * P:(fcp + 1) * 2 * P])
                for j in range(2):
                    nc.tensor.matmul(pu[j][:, :TOWN], lhsT=wl[:, j, :],
                                     rhs=Y2[ht], start=(ht == 0),
                                     stop=(ht == HT - 1))
            for j in range(2):
                sg = cpy_pool.tile([P, 512], BF16, tag="kc")
                nc.scalar.activation(out=sg[:, :TOWN], in_=pg[j][:, :TOWN],
                                     func=AFT.Silu)
                mt = mT_pool.tile([P, TOWN], BF16, tag="mT")
                mT.append(mt)
                nc.vector.tensor_mul(mt, sg[:, :TOWN], pu[j][:, :TOWN])

        for hcp in range(HT // 2):
            pd = [pmm_pool.tile([P, 512], F32, tag="mm", name=f"pd{j}") for j in range(2)]
            for fc in range(FC):
                wl = wlhs_pool.tile([P, 2, P], BF16, tag="wl")
                nc.sync.dma_start(
                    out=wl.rearrange("k a b -> k (a b)"),
                    in_=wdT[fc * P:(fc + 1) * P, hcp * 2 * P:(hcp + 1) * 2 * P])
                for j in range(2):
                    nc.tensor.matmul(pd[j][:, :TOWN], lhsT=wl[:, j, :],
                                     rhs=mT[fc], start=(fc == 0),
                                     stop=(fc == FC - 1))
            for j in range(2):
                hc = hcp * 2 + j
                dc = cpy_pool.tile([P, 512], BF16, tag="kc")
                nc.vector.tensor_copy(dc[:, :TOWN], pd[j][:, :TOWN])
                for qt in range(NT):
                    ptr = ptr_pool.tile([P, 2, P], BF16, tag="tr")
                    nc.tensor.transpose(ptr[:, 0, :],
                                        dc[:, qt * P:(qt + 1) * P], ident)
                    xb = x_tiles[qt][:, hc * P:(hc + 1) * P]
                    nc.vector.tensor_add(xb, xb, ptr[:, 0, :])

        for qt in range(NT):
            nc.sync.dma_start(out=y_out[qt], in_=x_tiles[qt])

    nc.compile()
    return nc


_CACHE = {}
LAST_RESULT = None


def _get_program(S_, FF_, ext, masked_items, n_mask):
    key = (S_, FF_, tuple(ext), tuple(sorted(masked_items)), n_mask)
    if key not in _CACHE:
        _CACHE[key] = _build_program(S_, FF_, tuple(ext), dict(masked_items),
                                     n_mask)
    return _CACHE[key]


def _prep_weights(q_w, k_w, v_w, o_w, gate_w, up_w, down_w, ln1_w, ln2_w):
    bf = ml_dtypes.bfloat16
    wqT = np.ascontiguousarray(
        (q_w * ln1_w[None, :]).T * (1.0 / math.sqrt(HD))).astype(bf)
    wkT = np.ascontiguousarray((k_w * ln1_w[None, :]).T).astype(bf)
    wvT = np.ascontiguousarray((v_w * ln1_w[None, :]).T).astype(bf)
    woT = np.ascontiguousarray(o_w.T).astype(bf)
    wgT = np.ascontiguousarray((gate_w * ln2_w[None, :]).T).astype(bf)
    wuT = np.ascontiguousarray((up_w * ln2_w[None, :]).T).astype(bf)
    wdT = np.ascontiguousarray(down_w.T).astype(bf)
    return wqT, wkT, wvT, woT, wgT, wuT, wdT


def _mask_structure(m, S_):
    """Derive the global (ext, masked) structure from the [S, S] mask.

    Returns per-q-tile bank extents, {(qt, bank) -> mask slot}, and the
    column-order table mapping (bank, position) -> global key column.
    """
    NT = S_ // TPG // P
    NB = S_ // 512
    col_of = np.empty((NB, 512), np.int64)
    for bi in range(NB):
        for o in range(TPG):
            col_of[bi, o * P:(o + 1) * P] = o + TPG * (P * bi + np.arange(P))
    need = np.zeros((NT, NB), bool)
    nonzero = np.zeros((NT, NB), bool)
    for r in range(TPG):
        for qt in range(NT):
            rows = r + TPG * (P * qt + np.arange(P))
            sub = m[rows]
            for bi in range(NB):
                blk = sub[:, col_of[bi]]
                need[qt, bi] |= bool((blk > MASK_CLAMP).any())
                nonzero[qt, bi] |= bool((blk < 0).any())
    ext = []
    masked = {}
    for qt in range(NT):
        e = int(np.max(np.nonzero(need[qt])[0])) + 1 if need[qt].any() else 1
        ext.append(e)
        for bi in range(e):
            if nonzero[qt, bi]:
                masked[(qt, bi)] = len(masked)
    return ext, masked, col_of


def kernel(hidden_states, attention_mask, q_w, k_w, v_w, o_w,
           gate_w, up_w, down_w, ln1_w, ln2_w):
    hidden_states = np.asarray(hidden_states, np.float32)
    m = np.maximum(np.asarray(attention_mask, np.float32)[0, 0], MASK_CLAMP)
    S_ = hidden_states.shape[1]
    FF_ = gate_w.shape[0]
    NT = S_ // TPG // P

    ext, masked, col_of = _mask_structure(m, S_)
    n_mask = len(masked)
    nc = _get_program(S_, FF_, ext, tuple(masked.items()), n_mask)

    wqT, wkT, wvT, woT, wgT, wuT, wdT = _prep_weights(
        np.asarray(q_w, np.float32), np.asarray(k_w, np.float32),
        np.asarray(v_w, np.float32), np.asarray(o_w, np.float32),
        np.asarray(gate_w, np.float32), np.asarray(up_w, np.float32),
        np.asarray(down_w, np.float32), np.asarray(ln1_w, np.float32),
        np.asarray(ln2_w, np.float32))

    in_maps = []
    for core in range(NC):
        b, r = core // TPG, core % TPG
        rows = r + TPG * np.arange(S_ // TPG)
        x_own = np.ascontiguousarray(
            hidden_states[b, rows].reshape(NT, P, H))
        mask_blocks = np.zeros((max(n_mask, 1), P, 512), np.float32)
        for (qt, bi), mi in masked.items():
            qrows = r + TPG * (P * qt + np.arange(P))
            mask_blocks[mi] = m[np.ix_(qrows, col_of[bi])]
        in_maps.append({
            "x": x_own, "mask": mask_blocks,
            "wqT": wqT, "wkT": wkT, "wvT": wvT, "woT": woT,
            "wgT": wgT, "wuT": wuT, "wdT": wdT,
        })

    res = run_bass_kernel_spmd(nc, in_maps, list(range(NC)),
                               trace=bool(os.environ.get("KERNEL_TRACE")))
    global LAST_RESULT
    LAST_RESULT = res

    out = np.empty((B, S_, H), np.float32)
    for core in range(NC):
        b, r = core // TPG, core % TPG
        rows = r + TPG * np.arange(S_ // TPG)
        out[b, rows] = res.results[core]["y"].reshape(S_ // TPG, H)
    return out



# revision 3
# speedup vs baseline: 1.3412x; 1.3412x over previous
"""Trainium2 Bass kernel for a dense GQA transformer layer (pre-norm, SwiGLU MLP).

Full shapes: B=2, S=2048, H=2048, NH=16, NKV=8, HD=128, FF=5632, fp32 I/O.

Sharding across 8 NeuronCores (one SPMD program):
  core = (b, r) with b = core//4 (data-parallel over batch),
  r = core%4 (sequence-parallel, row-interleaved: core owns rows r::4 of
  batch b). Row interleaving makes the causal-attention work identical on
  every core, which a single SPMD program requires.
  K/V are computed for owned rows only and AllGather'ed (groups of 4).
  Everything else (QKV/O projections, softmax, MLP) is token-parallel with
  full weights per core. Host reassembles the row-interleaved outputs.

Perf structure (v2):
  - All weights are pre-tiled host-side so each SBUF weight tile is one
    contiguous DRAM block (8KB per partition per DMA descriptor), loaded
    with a few large dma_starts through a single ring-buffered pool.
  - V weights are reused across token tiles and Q weights across q-tiles
    (loop order oc-outer) instead of being re-DMA'ed.
  - The K/V AllGather is split in four and each piece is issued as soon
    as its producer finishes, overlapping the gather with QKV compute.
  - Softmax normalization is folded into the P-transpose as a matmul with
    diag(1/den) instead of a full-width scalar rescale pass, and the
    scores for q-tile t+1 are emitted before transpose+AV of q-tile t so
    the PE never waits on the softmax chain.

Precision: bf16 matmuls with fp32 PSUM accumulation; softmax, norms and
residuals in fp32. RMSNorm weights are folded into the following projection
weights host-side; all weights are pre-transposed host-side to [in, out].
"""

import sys

if "/opt/trn_rl_repo" not in sys.path:
    sys.path.insert(0, "/opt/trn_rl_repo")

import math
import os
import numpy as np
import ml_dtypes

import concourse.bass as bass
import concourse.bacc as bacc
import concourse.tile as tile
import concourse.mybir as mybir
from concourse.bass_utils import run_bass_kernel_spmd
from concourse.masks import make_identity

F32 = mybir.dt.float32
BF16 = mybir.dt.bfloat16
AFT = mybir.ActivationFunctionType
ALU = mybir.AluOpType

# ---- fixed problem dims ----
B, S, H = 2, 2048, 2048
NH, NKV, HD = 16, 8, 128
FF = 5632
EPS = 1e-6
NC = 8          # cores
TPG = 4         # cores per batch group (sequence-parallel ways)
P = 128         # partitions

MASK_CLAMP = -30000.0


def _build_program(S_, FF_, ext, masked, n_mask):
    """Emit the SPMD program.

    S_: sequence length, FF_: mlp width (parameterized for small-scale tests)
    ext: tuple, per q-tile number of 512-col key banks to compute
    masked: dict {(qt, bank): mask_slot_index} for banks needing a mask add
    n_mask: number of [128, 512] mask blocks in the mask input
    """
    TOWN = S_ // TPG              # tokens owned per core
    NT = TOWN // P                # q-tiles per core
    NB = S_ // 512                # key banks (512 cols each)
    HT = H // P                   # 16 H tiles
    FC = FF_ // P                 # FF tiles
    FCP = FC // 2                 # FF pair tiles (gate/up psum pairs)
    KVH = NKV
    KH = KVH // 2
    assert len(ext) == NT
    QSCALE = 1.0 / math.sqrt(HD)

    nc = bacc.Bacc("TRN2", target_bir_lowering=False, debug=False,
                   num_devices=NC)

    # ---- I/O ----
    # weights arrive pre-tiled host-side; each [128, ...] tile is one
    # contiguous DRAM block per partition.
    x_in = nc.dram_tensor("x", [NT, P, H], F32, kind="ExternalInput").ap()
    wk_t = nc.dram_tensor("wk", [KH, P, HT, 256], BF16,
                          kind="ExternalInput").ap()
    wv_t = nc.dram_tensor("wv", [2, 2, P, HT // 2, 512], BF16,
                          kind="ExternalInput").ap()
    wq_t = nc.dram_tensor("wq", [4, 2, P, HT // 2, 512], BF16,
                          kind="ExternalInput").ap()
    wo_t = nc.dram_tensor("wo", [HT // 2, P, NH, 256], BF16,
                          kind="ExternalInput").ap()
    wg_t = nc.dram_tensor("wg", [FCP, P, HT, 256], BF16,
                          kind="ExternalInput").ap()
    wu_t = nc.dram_tensor("wu", [FCP, P, HT, 256], BF16,
                          kind="ExternalInput").ap()
    wd_t = nc.dram_tensor("wd", [HT // 2, 4, P, FC // 4, 256], BF16,
                          kind="ExternalInput").ap()
    mask_in = nc.dram_tensor("mask", [max(n_mask, 1), P, 512], BF16,
                             kind="ExternalInput").ap()
    y_out = nc.dram_tensor("y", [NT, P, H], F32, kind="ExternalOutput").ap()

    # ---- internal DRAM for the K/V AllGather (split in halves for overlap) ----
    k_loc = [nc.dram_tensor(f"k_loc{i}", [KH, HD, NT, P], BF16).ap()
             for i in range(2)]
    v_loc = [nc.dram_tensor(f"v_loc{i}", [NT, P, KH, HD], BF16).ap()
             for i in range(2)]
    k_all = [nc.dram_tensor(f"k_all{i}", [TPG, KH, HD, NT, P], BF16).ap()
             for i in range(2)]
    v_all = [nc.dram_tensor(f"v_all{i}", [TPG, NT, P, KH, HD], BF16).ap()
             for i in range(2)]

    groups = [[g * TPG + i for i in range(TPG)] for g in range(NC // TPG)]

    from contextlib import ExitStack
    with ExitStack() as ctx:
        tc = ctx.enter_context(tile.TileContext(nc))
        pool = lambda name, bufs, **kw: ctx.enter_context(
            tc.tile_pool(name=name, bufs=bufs, **kw))
        singles = pool("ones", 1)
        resid_pool = pool("resid", NT)
        ybuf = pool("ybuf", HT)
        qT_pool = pool("qTp", NT)
        kv_pool = pool("kvbuf", 2)
        ybf_pool = pool("ybfp", 1)
        pbf_pool = pool("pbf", 4)
        pT_pool = pool("pTp", 2)
        aT_pool = pool("aTp", 1)
        mT_pool = pool("mTp", FC)
        mask_pool = pool("maskp", max(n_mask, 1))
        small_pool = pool("small", 8)
        w_pool = pool("wp", 4)
        cpy_pool = pool("cpy", 4)
        ptr_pool = pool("ptr", 2, space="PSUM")
        pmm_pool = pool("pmm", 5, space="PSUM")
        psc_pool = pmm_pool
        pav_pool = pool("pav", 1, space="PSUM")

        ident = singles.tile([P, P], BF16)
        make_identity(nc, ident)
        eps_c = singles.tile([P, 1], F32)
        nc.vector.memset(eps_c, EPS)

        # mask blocks (bf16: only 0 / -30000, both exact; resident)
        mask_sb = []
        for mi in range(n_mask):
            mt = mask_pool.tile([P, 512], BF16, tag="mask")
            nc.sync.dma_start(out=mt, in_=mask_in[mi])
            mask_sb.append(mt)

        def rmsnorm_to_ybf(xt):
            ssum = small_pool.tile([P, 1], F32, tag="ss")
            ybf = ybf_pool.tile([P, H], BF16, tag="ybf")
            # ybf is first used as a throwaway output of the Square pass
            # (only the accumulator matters), then overwritten in place.
            nc.scalar.activation(out=ybf, in_=xt, func=AFT.Square,
                                 accum_out=ssum)
            std = small_pool.tile([P, 1], F32, tag="std")
            nc.scalar.activation(out=std, in_=ssum, func=AFT.Sqrt,
                                 scale=1.0 / H, bias=eps_c)
            rstd = small_pool.tile([P, 1], F32, tag="rstd")
            nc.vector.reciprocal(rstd, std)
            nc.scalar.activation(out=ybf, in_=xt, func=AFT.Copy, scale=rstd)
            return ybf

        def transpose_into(dst_tiles, ybf, qt):
            # ybf [128, H] (tokens x H) -> dst_tiles[ht][:, qt*P:...] (H x tok)
            for ht in range(HT):
                ptr = ptr_pool.tile([P, 2, P], BF16, tag="tr")
                nc.tensor.transpose(ptr[:, 0, :],
                                    ybf[:, ht * P:(ht + 1) * P], ident)
                nc.vector.tensor_copy(dst_tiles[ht][:, qt * P:(qt + 1) * P],
                                      ptr[:, 0, :])

        # Y[ht] : transposed normed activations [128 (H), TOWN] bf16
        Y = [ybuf.tile([P, TOWN], BF16, tag="y", name=f"Y{i}") for i in range(HT)]
        x_tiles = []

        # ---- stage A: load x, rmsnorm1, transpose into Y ----
        for qt in range(NT):
            xt = resid_pool.tile([P, H], F32, tag="x")
            x_tiles.append(xt)
            nc.sync.dma_start(out=xt, in_=x_in[qt])
        for qt in range(NT):
            ybf = rmsnorm_to_ybf(x_tiles[qt])
            transpose_into(Y, ybf, qt)

        # ---- stage B: K/V for owned tokens, split AllGather ASAP ----
        def k_half(half):
            for kvp in (2 * half, 2 * half + 1):   # kv head pairs
                wk_sb = w_pool.tile([P, HT, 256], BF16, tag="W", name="wk_sb")
                nc.sync.dma_start(out=wk_sb, in_=wk_t[kvp])
                pk = [pmm_pool.tile([P, 512], F32, tag="mm", name=f"pk{j}")
                      for j in range(2)]
                for ht in range(HT):
                    for j in range(2):
                        nc.tensor.matmul(
                            pk[j][:, :TOWN],
                            lhsT=wk_sb[:, ht, j * P:(j + 1) * P],
                            rhs=Y[ht], start=(ht == 0), stop=(ht == HT - 1))
                for j in range(2):
                    kvh = kvp * 2 + j
                    kc = cpy_pool.tile([P, 512], BF16, tag="kc")
                    nc.vector.tensor_copy(kc[:, :TOWN], pk[j][:, :TOWN])
                    nc.sync.dma_start(
                        out=k_loc[kvh // KH][kvh % KH].rearrange(
                            "d t i -> d (t i)"),
                        in_=kc[:, :TOWN])

        def v_half(half):
            wv_sb = []
            for hh in range(2):
                wt = w_pool.tile([P, HT // 2, 512], BF16, tag="W",
                                 name="wv_sb")
                nc.sync.dma_start(out=wt, in_=wv_t[half, hh])
                wv_sb.append(wt)
            for t in range(NT):
                pv = pmm_pool.tile([P, 512], F32, tag="mm")
                for ht in range(HT):
                    nc.tensor.matmul(
                        pv, lhsT=Y[ht][:, t * P:(t + 1) * P],
                        rhs=wv_sb[ht // (HT // 2)][:, ht % (HT // 2), :],
                        start=(ht == 0), stop=(ht == HT - 1))
                vc = cpy_pool.tile([P, 512], BF16, tag="kc")
                nc.vector.tensor_copy(vc, pv)
                nc.sync.dma_start(
                    out=v_loc[half][t].rearrange("s k d -> s (k d)"),
                    in_=vc)

        def gather(i, what):
            if what == "k":
                nc.gpsimd.collective_compute(
                    "AllGather", ALU.bypass, ins=[k_loc[i].opt()],
                    outs=[k_all[i].opt()], replica_groups=groups)
            else:
                nc.gpsimd.collective_compute(
                    "AllGather", ALU.bypass, ins=[v_loc[i].opt()],
                    outs=[v_all[i].opt()], replica_groups=groups)

        k_half(0)
        gather(0, "k")
        v_half(0)
        gather(0, "v")
        k_half(1)
        gather(1, "k")
        v_half(1)
        gather(1, "v")

        # ---- stage C: Q for owned tokens -> qT[qt] [128(d), NH, 128(tq)] ----
        # oc-outer so each weight tile is loaded once and reused for all qt.
        qT = [qT_pool.tile([P, NH, P], BF16, tag="qT", name=f"qT{i}")
              for i in range(NT)]
        for oc in range(NH * HD // 512):
            wq_sb = []
            for hh in range(2):
                wt = w_pool.tile([P, HT // 2, 512], BF16, tag="W",
                                 name="wq_sb")
                nc.sync.dma_start(out=wt, in_=wq_t[oc, hh])
                wq_sb.append(wt)
            for qt in range(NT):
                pq = pmm_pool.tile([P, 512], F32, tag="mm")
                for ht in range(HT):
                    nc.tensor.matmul(
                        pq, lhsT=Y[ht][:, qt * P:(qt + 1) * P],
                        rhs=wq_sb[ht // (HT // 2)][:, ht % (HT // 2), :],
                        start=(ht == 0), stop=(ht == HT - 1))
                qc = cpy_pool.tile([P, 512], BF16, tag="kc")
                nc.vector.tensor_copy(qc, pq)
                for hh in range(4):
                    ptr = ptr_pool.tile([P, 2, P], BF16, tag="tr")
                    nc.tensor.transpose(ptr[:, 0, :],
                                        qc[:, hh * P:(hh + 1) * P], ident)
                    nc.vector.tensor_copy(qT[qt][:, oc * 4 + hh, :],
                                          ptr[:, 0, :])

        # ---- stage D: attention ----
        aT = aT_pool.tile([P, NH, TOWN], BF16, tag="aT")
        for kvh in range(KVH):
            kT_sb = kv_pool.tile([P, NB, TPG, P], BF16, tag="kT")
            v_sb = kv_pool.tile([P, NB, TPG, HD], BF16, tag="vT")
            ka, va = k_all[kvh // KH], v_all[kvh // KH]
            for o in range(TPG):
                nc.sync.dma_start(out=kT_sb[:, :, o, :], in_=ka[o, kvh % KH])
                nc.sync.dma_start(
                    out=v_sb[:, :, o, :],
                    in_=va[o].rearrange("t s k d -> s t k d")[:, :, kvh % KH, :])

            def scores(qt):
                nbank = ext[qt]
                pb, dg = [], []
                for h2 in range(2):
                    h = 2 * kvh + h2
                    accs = small_pool.tile([P, NB], F32, tag="accs")
                    pbt = pbf_pool.tile([P, NB * 512], BF16, tag="pb")
                    pb.append(pbt)
                    for bi in range(nbank):
                        ps = psc_pool.tile([P, 512], F32, tag="mm")
                        nc.tensor.matmul(
                            ps, lhsT=qT[qt][:, h, :],
                            rhs=kT_sb[:, bi, :, :].rearrange(
                                "d o i -> d (o i)"),
                            start=True, stop=True)
                        mi = masked.get((qt, bi))
                        if mi is not None:
                            nc.vector.tensor_add(ps, ps, mask_sb[mi])
                        nc.scalar.activation(
                            out=pbt[:, bi * 512:(bi + 1) * 512], in_=ps,
                            func=AFT.Exp, accum_out=accs[:, bi:bi + 1])
                    den = small_pool.tile([P, 1], F32, tag="den")
                    nc.vector.tensor_reduce(den, accs[:, :nbank],
                                            mybir.AxisListType.X, ALU.add)
                    rec = small_pool.tile([P, 1], F32, tag="rec")
                    nc.vector.reciprocal(rec, den)
                    # diag(1/den): folded into the P-transpose matmul
                    dgt = small_pool.tile([P, P], BF16, tag="diag")
                    nc.vector.tensor_scalar_mul(dgt, ident, rec)
                    dg.append(dgt)
                return pb, dg

            def trans_av(qt, pb, dg):
                nbank = ext[qt]
                pav = pav_pool.tile([P, 2, P], F32, tag="av")
                for pc in range(nbank * TPG):
                    pT_ps = ptr_pool.tile([P, 2, P], F32, tag="tr")
                    for h2 in range(2):
                        nc.tensor.matmul(
                            pT_ps[:, h2, :],
                            lhsT=pb[h2][:, pc * P:(pc + 1) * P],
                            rhs=dg[h2], start=True, stop=True)
                    pT_sb = pT_pool.tile([P, 2, P], BF16, tag="pT")
                    nc.vector.tensor_copy(pT_sb, pT_ps)
                    nc.tensor.matmul(
                        pav, lhsT=v_sb[:, pc // TPG, pc % TPG, :],
                        rhs=pT_sb.rearrange("s h i -> s (h i)"),
                        start=(pc == 0), stop=(pc == nbank * TPG - 1))
                nc.vector.tensor_copy(
                    aT[:, 2 * kvh:2 * kvh + 2, qt * P:(qt + 1) * P], pav)

            prev = None
            for qt in range(NT):
                cur = (qt, *scores(qt))
                if prev is not None:
                    trans_av(*prev)
                prev = cur
            trans_av(*prev)

        # ---- stage E: O projection, streamed transpose + residual into x ----
        for hcp in range(HT // 2):         # H-column pairs
            wo_sb = w_pool.tile([P, NH, 256], BF16, tag="W", name="wo_sb")
            nc.sync.dma_start(out=wo_sb, in_=wo_t[hcp])
            po = [pmm_pool.tile([P, 512], F32, tag="mm", name=f"po{j}")
                  for j in range(2)]
            for h in range(NH):
                for j in range(2):
                    nc.tensor.matmul(po[j][:, :TOWN],
                                     lhsT=wo_sb[:, h, j * P:(j + 1) * P],
                                     rhs=aT[:, h, :], start=(h == 0),
                                     stop=(h == NH - 1))
            for j in range(2):
                hc = hcp * 2 + j
                oc_ = cpy_pool.tile([P, 512], BF16, tag="kc")
                nc.vector.tensor_copy(oc_[:, :TOWN], po[j][:, :TOWN])
                for qt in range(NT):
                    ptr = ptr_pool.tile([P, 2, P], BF16, tag="tr")
                    nc.tensor.transpose(ptr[:, 0, :],
                                        oc_[:, qt * P:(qt + 1) * P], ident)
                    xb = x_tiles[qt][:, hc * P:(hc + 1) * P]
                    nc.vector.tensor_add(xb, xb, ptr[:, 0, :])

        # ---- rmsnorm2 -> Y2 ----
        Y2 = [ybuf.tile([P, TOWN], BF16, tag="y", name=f"Y2_{i}")
              for i in range(HT)]
        for qt in range(NT):
            ybf = rmsnorm_to_ybf(x_tiles[qt])
            transpose_into(Y2, ybf, qt)

        # ---- stage F: MLP ----
        mT = []
        for fcp in range(FCP):             # FF-tile pairs
            wg_sb = w_pool.tile([P, HT, 256], BF16, tag="W", name="wg_sb")
            nc.sync.dma_start(out=wg_sb, in_=wg_t[fcp])
            wu_sb = w_pool.tile([P, HT, 256], BF16, tag="W", name="wu_sb")
            nc.sync.dma_start(out=wu_sb, in_=wu_t[fcp])
            pg = [pmm_pool.tile([P, 512], F32, tag="mm", name=f"pg{j}")
                  for j in range(2)]
            for ht in range(HT):
                for j in range(2):
                    nc.tensor.matmul(pg[j][:, :TOWN],
                                     lhsT=wg_sb[:, ht, j * P:(j + 1) * P],
                                     rhs=Y2[ht], start=(ht == 0),
                                     stop=(ht == HT - 1))
            pu = [psc_pool.tile([P, 512], F32, tag="mm", name=f"pu{j}")
                  for j in range(2)]
            for ht in range(HT):
                for j in range(2):
                    nc.tensor.matmul(pu[j][:, :TOWN],
                                     lhsT=wu_sb[:, ht, j * P:(j + 1) * P],
                                     rhs=Y2[ht], start=(ht == 0),
                                     stop=(ht == HT - 1))
            for j in range(2):
                sg = cpy_pool.tile([P, 512], BF16, tag="kc")
                nc.scalar.activation(out=sg[:, :TOWN], in_=pg[j][:, :TOWN],
                                     func=AFT.Silu)
                mt = mT_pool.tile([P, TOWN], BF16, tag="mT")
                mT.append(mt)
                nc.vector.tensor_mul(mt, sg[:, :TOWN], pu[j][:, :TOWN])

        FQ = FC // 4
        for hcp in range(HT // 2):
            wd_sb = []
            for q in range(4):
                wt = w_pool.tile([P, FQ, 256], BF16, tag="W", name="wd_sb")
                nc.sync.dma_start(out=wt, in_=wd_t[hcp, q])
                wd_sb.append(wt)
            pd = [pmm_pool.tile([P, 512], F32, tag="mm", name=f"pd{j}")
                  for j in range(2)]
            for fc in range(FC):
                for j in range(2):
                    nc.tensor.matmul(
                        pd[j][:, :TOWN],
                        lhsT=wd_sb[fc // FQ][:, fc % FQ, j * P:(j + 1) * P],
                        rhs=mT[fc], start=(fc == 0), stop=(fc == FC - 1))
            for j in range(2):
                hc = hcp * 2 + j
                dc = cpy_pool.tile([P, 512], BF16, tag="kc")
                nc.vector.tensor_copy(dc[:, :TOWN], pd[j][:, :TOWN])
                for qt in range(NT):
                    ptr = ptr_pool.tile([P, 2, P], BF16, tag="tr")
                    nc.tensor.transpose(ptr[:, 0, :],
                                        dc[:, qt * P:(qt + 1) * P], ident)
                    xb = x_tiles[qt][:, hc * P:(hc + 1) * P]
                    nc.vector.tensor_add(xb, xb, ptr[:, 0, :])

        for qt in range(NT):
            nc.sync.dma_start(out=y_out[qt], in_=x_tiles[qt])

    nc.compile()
    return nc


_CACHE = {}
LAST_RESULT = None


def _get_program(S_, FF_, ext, masked_items, n_mask):
    key = (S_, FF_, tuple(ext), tuple(sorted(masked_items)), n_mask)
    if key not in _CACHE:
        _CACHE[key] = _build_program(S_, FF_, tuple(ext), dict(masked_items),
                                     n_mask)
    return _CACHE[key]


def _prep_weights(q_w, k_w, v_w, o_w, gate_w, up_w, down_w, ln1_w, ln2_w):
    bf = ml_dtypes.bfloat16
    HT = H // P
    FF_ = gate_w.shape[0]
    FC = FF_ // P
    FCP = FC // 2
    FQ = FC // 4
    wqT = ((q_w * ln1_w[None, :]).T * (1.0 / math.sqrt(HD))).astype(np.float32)
    wkT = ((k_w * ln1_w[None, :]).T).astype(np.float32)
    wvT = ((v_w * ln1_w[None, :]).T).astype(np.float32)
    woT = o_w.T.astype(np.float32)
    wgT = ((gate_w * ln2_w[None, :]).T).astype(np.float32)
    wuT = ((up_w * ln2_w[None, :]).T).astype(np.float32)
    wdT = down_w.T.astype(np.float32)

    c = np.ascontiguousarray
    # tiled layouts (one contiguous DRAM block per SBUF partition per tile)
    wk_t = c(wkT.reshape(HT, P, NKV // 2, 256)
             .transpose(2, 1, 0, 3)).astype(bf)               # [4,128,16,256]
    wv_t = c(wvT.reshape(2, HT // 2, P, 2, 512)
             .transpose(3, 0, 2, 1, 4)).astype(bf)            # [2,2,128,8,512]
    wq_t = c(wqT.reshape(2, HT // 2, P, 4, 512)
             .transpose(3, 0, 2, 1, 4)).astype(bf)            # [4,2,128,8,512]
    wo_t = c(woT.reshape(NH, P, HT // 2, 256)
             .transpose(2, 1, 0, 3)).astype(bf)               # [8,128,16,256]
    wg_t = c(wgT.reshape(HT, P, FCP, 256)
             .transpose(2, 1, 0, 3)).astype(bf)               # [22,128,16,256]
    wu_t = c(wuT.reshape(HT, P, FCP, 256)
             .transpose(2, 1, 0, 3)).astype(bf)
    wd_t = c(wdT.reshape(4, FQ, P, HT // 2, 256)
             .transpose(3, 0, 2, 1, 4)).astype(bf)            # [8,4,128,11,256]
    return wk_t, wv_t, wq_t, wo_t, wg_t, wu_t, wd_t


def _mask_structure(m, S_):
    """Derive the global (ext, masked) structure from the [S, S] mask.

    Returns per-q-tile bank extents, {(qt, bank) -> mask slot}, and the
    column-order table mapping (bank, position) -> global key column.
    """
    NT = S_ // TPG // P
    NB = S_ // 512
    col_of = np.empty((NB, 512), np.int64)
    for bi in range(NB):
        for o in range(TPG):
            col_of[bi, o * P:(o + 1) * P] = o + TPG * (P * bi + np.arange(P))
    need = np.zeros((NT, NB), bool)
    nonzero = np.zeros((NT, NB), bool)
    for r in range(TPG):
        for qt in range(NT):
            rows = r + TPG * (P * qt + np.arange(P))
            sub = m[rows]
            for bi in range(NB):
                blk = sub[:, col_of[bi]]
                need[qt, bi] |= bool((blk > MASK_CLAMP).any())
                nonzero[qt, bi] |= bool((blk < 0).any())
    ext = []
    masked = {}
    for qt in range(NT):
        e = int(np.max(np.nonzero(need[qt])[0])) + 1 if need[qt].any() else 1
        ext.append(e)
        for bi in range(e):
            if nonzero[qt, bi]:
                masked[(qt, bi)] = len(masked)
    return ext, masked, col_of


def kernel(hidden_states, attention_mask, q_w, k_w, v_w, o_w,
           gate_w, up_w, down_w, ln1_w, ln2_w):
    hidden_states = np.asarray(hidden_states, np.float32)
    m = np.maximum(np.asarray(attention_mask, np.float32)[0, 0], MASK_CLAMP)
    S_ = hidden_states.shape[1]
    FF_ = gate_w.shape[0]
    NT = S_ // TPG // P

    ext, masked, col_of = _mask_structure(m, S_)
    n_mask = len(masked)
    nc = _get_program(S_, FF_, ext, tuple(masked.items()), n_mask)

    wk_t, wv_t, wq_t, wo_t, wg_t, wu_t, wd_t = _prep_weights(
        np.asarray(q_w, np.float32), np.asarray(k_w, np.float32),
        np.asarray(v_w, np.float32), np.asarray(o_w, np.float32),
        np.asarray(gate_w, np.float32), np.asarray(up_w, np.float32),
        np.asarray(down_w, np.float32), np.asarray(ln1_w, np.float32),
        np.asarray(ln2_w, np.float32))

    bf = ml_dtypes.bfloat16
    in_maps = []
    for core in range(NC):
        b, r = core // TPG, core % TPG
        rows = r + TPG * np.arange(S_ // TPG)
        x_own = np.ascontiguousarray(
            hidden_states[b, rows].reshape(NT, P, H))
        mask_blocks = np.zeros((max(n_mask, 1), P, 512), np.float32)
        for (qt, bi), mi in masked.items():
            qrows = r + TPG * (P * qt + np.arange(P))
            mask_blocks[mi] = m[np.ix_(qrows, col_of[bi])]
        in_maps.append({
            "x": x_own, "mask": mask_blocks.astype(bf),
            "wk": wk_t, "wv": wv_t, "wq": wq_t, "wo": wo_t,
            "wg": wg_t, "wu": wu_t, "wd": wd_t,
        })

    res = run_bass_kernel_spmd(nc, in_maps, list(range(NC)),
                               trace=bool(os.environ.get("KERNEL_TRACE")))
    global LAST_RESULT
    LAST_RESULT = res

    out = np.empty((B, S_, H), np.float32)
    for core in range(NC):
        b, r = core // TPG, core % TPG
        rows = r + TPG * np.arange(S_ // TPG)
        out[b, rows] = res.results[core]["y"].reshape(S_ // TPG, H)
    return out


# revision 8
# speedup vs baseline: 1.4302x; 1.0663x over previous
"""Trainium2 Bass kernel for a dense GQA transformer layer (pre-norm, SwiGLU MLP).

Full shapes: B=2, S=2048, H=2048, NH=16, NKV=8, HD=128, FF=5632, fp32 I/O.

Sharding across 8 NeuronCores (one SPMD program):
  core = (b, r) with b = core//4 (data-parallel over batch),
  r = core%4 (sequence-parallel, row-interleaved: core owns rows r::4 of
  batch b). Row interleaving makes the causal-attention work identical on
  every core, which a single SPMD program requires.
  K/V are computed for owned rows only and AllGather'ed (groups of 4).
  Everything else (QKV/O projections, softmax, MLP) is token-parallel with
  full weights per core. Host reassembles the row-interleaved outputs.

Perf structure (v2):
  - All weights are pre-tiled host-side so each SBUF weight tile is one
    contiguous DRAM block (8KB per partition per DMA descriptor), loaded
    with a few large dma_starts through a single ring-buffered pool.
  - V weights are reused across token tiles and Q weights across q-tiles
    (loop order oc-outer) instead of being re-DMA'ed.
  - The K/V AllGather is split in four and each piece is issued as soon
    as its producer finishes, overlapping the gather with QKV compute.
  - Softmax normalization is folded into the P-transpose as a matmul with
    diag(1/den) instead of a full-width scalar rescale pass, and the
    scores for q-tile t+1 are emitted before transpose+AV of q-tile t so
    the PE never waits on the softmax chain.

Precision: bf16 matmuls with fp32 PSUM accumulation; softmax, norms and
residuals in fp32. RMSNorm weights are folded into the following projection
weights host-side; all weights are pre-transposed host-side to [in, out].
"""

import sys

if "/opt/trn_rl_repo" not in sys.path:
    sys.path.insert(0, "/opt/trn_rl_repo")

import math
import os
import numpy as np
import ml_dtypes

import concourse.bass as bass
import concourse.bacc as bacc
import concourse.tile as tile
import concourse.mybir as mybir
from concourse.bass_utils import run_bass_kernel_spmd
from concourse.masks import make_identity

F32 = mybir.dt.float32
BF16 = mybir.dt.bfloat16
AFT = mybir.ActivationFunctionType
ALU = mybir.AluOpType

# ---- fixed problem dims ----
B, S, H = 2, 2048, 2048
NH, NKV, HD = 16, 8, 128
FF = 5632
EPS = 1e-6
NC = 8          # cores
TPG = 4         # cores per batch group (sequence-parallel ways)
P = 128         # partitions

MASK_CLAMP = -30000.0


def _build_program(S_, FF_, ext, masked, n_mask):
    """Emit the SPMD program.

    S_: sequence length, FF_: mlp width (parameterized for small-scale tests)
    ext: tuple, per q-tile number of 512-col key banks to compute
    masked: dict {(qt, bank): mask_slot_index} for banks needing a mask add
    n_mask: number of [128, 512] mask blocks in the mask input
    """
    TOWN = S_ // TPG              # tokens owned per core
    NT = TOWN // P                # q-tiles per core
    NB = S_ // 512                # key banks (512 cols each)
    HT = H // P                   # 16 H tiles
    FC = FF_ // P                 # FF tiles
    FCP = FC // 2                 # FF pair tiles (gate/up psum pairs)
    KVH = NKV
    KH = KVH // 2
    assert len(ext) == NT
    QSCALE = 1.0 / math.sqrt(HD)

    nc = bacc.Bacc("TRN2", target_bir_lowering=False, debug=False,
                   num_devices=NC)

    # ---- I/O ----
    # weights arrive pre-tiled host-side; each [128, ...] tile is one
    # contiguous DRAM block per partition.
    x_in = nc.dram_tensor("x", [NT, P, H], F32, kind="ExternalInput").ap()
    wk_t = nc.dram_tensor("wk", [KH, P, HT, 256], BF16,
                          kind="ExternalInput").ap()
    wv_t = nc.dram_tensor("wv", [2, 2, P, HT // 2, 512], BF16,
                          kind="ExternalInput").ap()
    wq_t = nc.dram_tensor("wq", [4, 2, P, HT // 2, 512], BF16,
                          kind="ExternalInput").ap()
    wo_t = nc.dram_tensor("wo", [HT // 2, P, NH, 256], BF16,
                          kind="ExternalInput").ap()
    wg_t = nc.dram_tensor("wg", [FCP, P, HT, 256], BF16,
                          kind="ExternalInput").ap()
    wu_t = nc.dram_tensor("wu", [FCP, P, HT, 256], BF16,
                          kind="ExternalInput").ap()
    wd_t = nc.dram_tensor("wd", [HT // 2, 4, P, FC // 4, 256], BF16,
                          kind="ExternalInput").ap()
    mask_in = nc.dram_tensor("mask", [max(n_mask, 1), P, 512], BF16,
                             kind="ExternalInput").ap()
    y_out = nc.dram_tensor("y", [NT, P, H], F32, kind="ExternalOutput").ap()

    # ---- internal DRAM for the K/V AllGather (split in halves for overlap) ----
    k_loc = [nc.dram_tensor(f"k_loc{i}", [KH, HD, NT, P], BF16).ap()
             for i in range(2)]
    v_loc = [nc.dram_tensor(f"v_loc{i}", [NT, P, KH, HD], BF16).ap()
             for i in range(2)]
    k_all = [nc.dram_tensor(f"k_all{i}", [TPG, KH, HD, NT, P], BF16).ap()
             for i in range(2)]
    v_all = [nc.dram_tensor(f"v_all{i}", [TPG, NT, P, KH, HD], BF16).ap()
             for i in range(2)]

    groups = [[g * TPG + i for i in range(TPG)] for g in range(NC // TPG)]

    from contextlib import ExitStack
    with ExitStack() as ctx:
        tc = ctx.enter_context(tile.TileContext(nc))
        pool = lambda name, bufs, **kw: ctx.enter_context(
            tc.tile_pool(name=name, bufs=bufs, **kw))
        singles = pool("ones", 1)
        resid_pool = pool("resid", NT)
        ybuf = pool("ybuf", HT)
        qT_pool = pool("qTp", NT)
        kv_pool = pool("kvbuf", 2)
        ybf_pool = pool("ybfp", 1)
        pbf_pool = pool("pbf", 4)
        pT_pool = pool("pTp", 4)
        aT_pool = pool("aTp", 1)
        mT_pool = pool("mTp", FC)
        mask_pool = pool("maskp", max(n_mask, 1))
        small_pool = pool("small", 8)
        w_pool = pool("wp", 4)
        cpy_pool = pool("cpy", 6)
        ptr_pool = pool("ptr", 2, space="PSUM")
        pmm_pool = pool("pmm", 5, space="PSUM")
        psc_pool = pmm_pool
        pav_pool = pool("pav", 1, space="PSUM")

        ident = singles.tile([P, P], BF16)
        make_identity(nc, ident)
        eps_c = singles.tile([P, 1], F32)
        nc.vector.memset(eps_c, EPS)

        # mask blocks (bf16: only 0 / -30000, both exact; resident)
        mask_sb = []
        for mi in range(n_mask):
            mt = mask_pool.tile([P, 512], BF16, tag="mask")
            nc.sync.dma_start(out=mt, in_=mask_in[mi])
            mask_sb.append(mt)

        def rmsnorm_to_ybf(xt):
            ssum = small_pool.tile([P, 1], F32, tag="ss")
            ybf = ybf_pool.tile([P, H], BF16, tag="ybf")
            # ybf is first used as a throwaway output of the Square pass
            # (only the accumulator matters), then overwritten in place.
            nc.scalar.activation(out=ybf, in_=xt, func=AFT.Square,
                                 accum_out=ssum)
            std = small_pool.tile([P, 1], F32, tag="std")
            nc.scalar.activation(out=std, in_=ssum, func=AFT.Sqrt,
                                 scale=1.0 / H, bias=eps_c)
            rstd = small_pool.tile([P, 1], F32, tag="rstd")
            nc.vector.reciprocal(rstd, std)
            nc.scalar.activation(out=ybf, in_=xt, func=AFT.Copy, scale=rstd)
            return ybf

        def transpose_into(dst_tiles, ybf, qt):
            # ybf [128, H] (tokens x H) -> dst_tiles[ht][:, qt*P:...] (H x tok)
            for ht in range(HT):
                ptr = ptr_pool.tile([P, 2, P], BF16, tag="tr")
                nc.tensor.transpose(ptr[:, 0, :],
                                    ybf[:, ht * P:(ht + 1) * P], ident)
                nc.vector.tensor_copy(dst_tiles[ht][:, qt * P:(qt + 1) * P],
                                      ptr[:, 0, :])

        # Y[ht] : transposed normed activations [128 (H), TOWN] bf16
        Y = [ybuf.tile([P, TOWN], BF16, tag="y", name=f"Y{i}") for i in range(HT)]
        x_tiles = []

        # ---- stage A: load x, rmsnorm1, transpose into Y ----
        for qt in range(NT):
            xt = resid_pool.tile([P, H], F32, tag="x")
            x_tiles.append(xt)
            nc.sync.dma_start(out=xt, in_=x_in[qt])
        for qt in range(NT):
            ybf = rmsnorm_to_ybf(x_tiles[qt])
            transpose_into(Y, ybf, qt)

        # ---- stage B: K/V for owned tokens, split AllGather ASAP ----
        def k_half(half):
            for kvp in (2 * half, 2 * half + 1):   # kv head pairs
                wk_sb = w_pool.tile([P, HT, 256], BF16, tag="W", name="wk_sb")
                nc.sync.dma_start(out=wk_sb, in_=wk_t[kvp])
                pk = [pmm_pool.tile([P, 512], F32, tag="mm", name=f"pk{j}")
                      for j in range(2)]
                for ht in range(HT):
                    for j in range(2):
                        nc.tensor.matmul(
                            pk[j][:, :TOWN],
                            lhsT=wk_sb[:, ht, j * P:(j + 1) * P],
                            rhs=Y[ht], start=(ht == 0), stop=(ht == HT - 1))
                for j in range(2):
                    kvh = kvp * 2 + j
                    kc = cpy_pool.tile([P, 512], BF16, tag="kc")
                    nc.vector.tensor_copy(kc[:, :TOWN], pk[j][:, :TOWN])
                    nc.sync.dma_start(
                        out=k_loc[kvh // KH][kvh % KH].rearrange(
                            "d t i -> d (t i)"),
                        in_=kc[:, :TOWN])

        def v_half(half):
            wv_sb = []
            for hh in range(2):
                wt = w_pool.tile([P, HT // 2, 512], BF16, tag="W",
                                 name="wv_sb")
                nc.sync.dma_start(out=wt, in_=wv_t[half, hh])
                wv_sb.append(wt)
            for t in range(NT):
                pv = pmm_pool.tile([P, 512], F32, tag="mm")
                for ht in range(HT):
                    nc.tensor.matmul(
                        pv, lhsT=Y[ht][:, t * P:(t + 1) * P],
                        rhs=wv_sb[ht // (HT // 2)][:, ht % (HT // 2), :],
                        start=(ht == 0), stop=(ht == HT - 1))
                vc = cpy_pool.tile([P, 512], BF16, tag="kc")
                nc.vector.tensor_copy(vc, pv)
                nc.sync.dma_start(
                    out=v_loc[half][t].rearrange("s k d -> s (k d)"),
                    in_=vc)

        def gather(i, what):
            if what == "k":
                nc.gpsimd.collective_compute(
                    "AllGather", ALU.bypass, ins=[k_loc[i].opt()],
                    outs=[k_all[i].opt()], replica_groups=groups)
            else:
                nc.gpsimd.collective_compute(
                    "AllGather", ALU.bypass, ins=[v_loc[i].opt()],
                    outs=[v_all[i].opt()], replica_groups=groups)

        k_half(0)
        gather(0, "k")
        v_half(0)
        gather(0, "v")
        k_half(1)
        gather(1, "k")
        v_half(1)
        gather(1, "v")

        # ---- stage C: Q for owned tokens -> qT[qt] [128(d), NH, 128(tq)] ----
        # oc-outer so each weight tile is loaded once and reused for all qt.
        qT = [qT_pool.tile([P, NH, P], BF16, tag="qT", name=f"qT{i}")
              for i in range(NT)]
        for oc in range(NH * HD // 512):
            wq_sb = []
            for hh in range(2):
                wt = w_pool.tile([P, HT // 2, 512], BF16, tag="W",
                                 name="wq_sb")
                nc.sync.dma_start(out=wt, in_=wq_t[oc, hh])
                wq_sb.append(wt)
            for qt in range(NT):
                pq = pmm_pool.tile([P, 512], F32, tag="mm")
                for ht in range(HT):
                    nc.tensor.matmul(
                        pq, lhsT=Y[ht][:, qt * P:(qt + 1) * P],
                        rhs=wq_sb[ht // (HT // 2)][:, ht % (HT // 2), :],
                        start=(ht == 0), stop=(ht == HT - 1))
                qc = cpy_pool.tile([P, 512], BF16, tag="kc")
                nc.vector.tensor_copy(qc, pq)
                for hh in range(4):
                    ptr = ptr_pool.tile([P, 2, P], BF16, tag="tr")
                    nc.tensor.transpose(ptr[:, 0, :],
                                        qc[:, hh * P:(hh + 1) * P], ident)
                    nc.vector.tensor_copy(qT[qt][:, oc * 4 + hh, :],
                                          ptr[:, 0, :])

        # ---- stage D: attention ----
        aT = aT_pool.tile([P, NH, TOWN], BF16, tag="aT")
        for kvh in range(KVH):
            kT_sb = kv_pool.tile([P, NB, TPG, P], BF16, tag="kT")
            v_sb = kv_pool.tile([P, NB, TPG, HD], BF16, tag="vT")
            ka, va = k_all[kvh // KH], v_all[kvh // KH]
            # SWDGE (gpsimd queue): these waits depend on the collectives,
            # and on the sync queue they head-of-line block later weight DMAs.
            for o in range(TPG):
                nc.gpsimd.dma_start(out=kT_sb[:, :, o, :], in_=ka[o, kvh % KH])
                nc.gpsimd.dma_start(
                    out=v_sb[:, :, o, :],
                    in_=va[o].rearrange("t s k d -> s t k d")[:, :, kvh % KH, :])

            def scores(qt):
                nbank = ext[qt]
                pb, dg = [], []
                for h2 in range(2):
                    h = 2 * kvh + h2
                    accs = small_pool.tile([P, NB], F32, tag="accs")
                    pbt = pbf_pool.tile([P, NB * 512], BF16, tag="pb")
                    pb.append(pbt)
                    for bi in range(nbank):
                        ps = psc_pool.tile([P, 512], F32, tag="mm")
                        nc.tensor.matmul(
                            ps, lhsT=qT[qt][:, h, :],
                            rhs=kT_sb[:, bi, :, :].rearrange(
                                "d o i -> d (o i)"),
                            start=True, stop=True)
                        mi = masked.get((qt, bi))
                        if mi is not None:
                            nc.vector.tensor_add(ps, ps, mask_sb[mi])
                        nc.scalar.activation(
                            out=pbt[:, bi * 512:(bi + 1) * 512], in_=ps,
                            func=AFT.Exp, accum_out=accs[:, bi:bi + 1])
                    den = small_pool.tile([P, 1], F32, tag="den")
                    nc.vector.tensor_reduce(den, accs[:, :nbank],
                                            mybir.AxisListType.X, ALU.add)
                    rec = small_pool.tile([P, 1], F32, tag="rec")
                    nc.vector.reciprocal(rec, den)
                    # diag(1/den): folded into the P-transpose matmul
                    dgt = small_pool.tile([P, P], BF16, tag="diag")
                    nc.vector.tensor_scalar_mul(dgt, ident, rec)
                    dg.append(dgt)
                return pb, dg

            def trans_av(qt, pb, dg):
                # P-transposes run one chunk ahead of the AV accumulation so
                # the (in-order) PE never waits on the psum->sbuf copy.
                nbank = ext[qt]
                pav = pav_pool.tile([P, 2, P], F32, tag="av")
                npc = nbank * TPG
                pT_sbs = []
                for pc in range(npc):
                    pT_ps = ptr_pool.tile([P, 2, P], F32, tag="tr")
                    for h2 in range(2):
                        nc.tensor.matmul(
                            pT_ps[:, h2, :],
                            lhsT=pb[h2][:, pc * P:(pc + 1) * P],
                            rhs=dg[h2], start=True, stop=True)
                    pT_sb = pT_pool.tile([P, 2, P], BF16, tag="pT")
                    nc.vector.tensor_copy(pT_sb, pT_ps)
                    pT_sbs.append(pT_sb)
                    if pc >= 1:
                        nc.tensor.matmul(
                            pav, lhsT=v_sb[:, (pc - 1) // TPG, (pc - 1) % TPG, :],
                            rhs=pT_sbs[pc - 1].rearrange("s h i -> s (h i)"),
                            start=(pc - 1 == 0), stop=False)
                nc.tensor.matmul(
                    pav, lhsT=v_sb[:, (npc - 1) // TPG, (npc - 1) % TPG, :],
                    rhs=pT_sbs[npc - 1].rearrange("s h i -> s (h i)"),
                    start=(npc == 1), stop=True)
                nc.vector.tensor_copy(
                    aT[:, 2 * kvh:2 * kvh + 2, qt * P:(qt + 1) * P], pav)

            prev = None
            for qt in range(NT):
                cur = (qt, *scores(qt))
                if prev is not None:
                    trans_av(*prev)
                prev = cur
            trans_av(*prev)

        # ---- stage E: O projection, streamed transpose + residual into x ----
        for hcp in range(HT // 2):         # H-column pairs
            wo_sb = w_pool.tile([P, NH, 256], BF16, tag="W", name="wo_sb")
            nc.sync.dma_start(out=wo_sb, in_=wo_t[hcp])
            po = [pmm_pool.tile([P, 512], F32, tag="mm", name=f"po{j}")
                  for j in range(2)]
            for h in range(NH):
                for j in range(2):
                    nc.tensor.matmul(po[j][:, :TOWN],
                                     lhsT=wo_sb[:, h, j * P:(j + 1) * P],
                                     rhs=aT[:, h, :], start=(h == 0),
                                     stop=(h == NH - 1))
            for j in range(2):
                hc = hcp * 2 + j
                oc_ = cpy_pool.tile([P, 512], BF16, tag="kc")
                nc.vector.tensor_copy(oc_[:, :TOWN], po[j][:, :TOWN])
                for qt in range(NT):
                    ptr = ptr_pool.tile([P, 2, P], BF16, tag="tr")
                    nc.tensor.transpose(ptr[:, 0, :],
                                        oc_[:, qt * P:(qt + 1) * P], ident)
                    xb = x_tiles[qt][:, hc * P:(hc + 1) * P]
                    nc.vector.tensor_add(xb, xb, ptr[:, 0, :])

        # ---- rmsnorm2 -> Y2 ----
        Y2 = [ybuf.tile([P, TOWN], BF16, tag="y", name=f"Y2_{i}")
              for i in range(HT)]
        for qt in range(NT):
            ybf = rmsnorm_to_ybf(x_tiles[qt])
            transpose_into(Y2, ybf, qt)

        # ---- stage F: MLP ----
        mT = []
        for fcp in range(FCP):             # FF-tile pairs
            wg_sb = w_pool.tile([P, HT, 256], BF16, tag="W", name="wg_sb")
            nc.sync.dma_start(out=wg_sb, in_=wg_t[fcp])
            wu_sb = w_pool.tile([P, HT, 256], BF16, tag="W", name="wu_sb")
            nc.sync.dma_start(out=wu_sb, in_=wu_t[fcp])
            pg = [pmm_pool.tile([P, 512], F32, tag="mm", name=f"pg{j}")
                  for j in range(2)]
            for ht in range(HT):
                for j in range(2):
                    nc.tensor.matmul(pg[j][:, :TOWN],
                                     lhsT=wg_sb[:, ht, j * P:(j + 1) * P],
                                     rhs=Y2[ht], start=(ht == 0),
                                     stop=(ht == HT - 1))
            pu = [psc_pool.tile([P, 512], F32, tag="mm", name=f"pu{j}")
                  for j in range(2)]
            for ht in range(HT):
                for j in range(2):
                    nc.tensor.matmul(pu[j][:, :TOWN],
                                     lhsT=wu_sb[:, ht, j * P:(j + 1) * P],
                                     rhs=Y2[ht], start=(ht == 0),
                                     stop=(ht == HT - 1))
            for j in range(2):
                sg = cpy_pool.tile([P, 512], BF16, tag="kc")
                nc.scalar.activation(out=sg[:, :TOWN], in_=pg[j][:, :TOWN],
                                     func=AFT.Silu)
                mt = mT_pool.tile([P, TOWN], BF16, tag="mT")
                mT.append(mt)
                nc.vector.tensor_mul(mt, sg[:, :TOWN], pu[j][:, :TOWN])

        FQ = FC // 4
        for hcp in range(HT // 2):
            wd_sb = []
            for q in range(4):
                wt = w_pool.tile([P, FQ, 256], BF16, tag="W", name="wd_sb")
                nc.sync.dma_start(out=wt, in_=wd_t[hcp, q])
                wd_sb.append(wt)
            pd = [pmm_pool.tile([P, 512], F32, tag="mm", name=f"pd{j}")
                  for j in range(2)]
            for fc in range(FC):
                for j in range(2):
                    nc.tensor.matmul(
                        pd[j][:, :TOWN],
                        lhsT=wd_sb[fc // FQ][:, fc % FQ, j * P:(j + 1) * P],
                        rhs=mT[fc], start=(fc == 0), stop=(fc == FC - 1))
            for j in range(2):
                hc = hcp * 2 + j
                dc = cpy_pool.tile([P, 512], BF16, tag="kc")
                nc.vector.tensor_copy(dc[:, :TOWN], pd[j][:, :TOWN])
                for qt in range(NT):
                    ptr = ptr_pool.tile([P, 2, P], BF16, tag="tr")
                    nc.tensor.transpose(ptr[:, 0, :],
                                        dc[:, qt * P:(qt + 1) * P], ident)
                    xb = x_tiles[qt][:, hc * P:(hc + 1) * P]
                    nc.vector.tensor_add(xb, xb, ptr[:, 0, :])

        for qt in range(NT):
            nc.sync.dma_start(out=y_out[qt], in_=x_tiles[qt])

    nc.compile()
    return nc


_CACHE = {}
LAST_RESULT = None


def _get_program(S_, FF_, ext, masked_items, n_mask):
    key = (S_, FF_, tuple(ext), tuple(sorted(masked_items)), n_mask)
    if key not in _CACHE:
        _CACHE[key] = _build_program(S_, FF_, tuple(ext), dict(masked_items),
                                     n_mask)
    return _CACHE[key]


def _prep_weights(q_w, k_w, v_w, o_w, gate_w, up_w, down_w, ln1_w, ln2_w):
    bf = ml_dtypes.bfloat16
    HT = H // P
    FF_ = gate_w.shape[0]
    FC = FF_ // P
    FCP = FC // 2
    FQ = FC // 4
    wqT = ((q_w * ln1_w[None, :]).T * (1.0 / math.sqrt(HD))).astype(np.float32)
    wkT = ((k_w * ln1_w[None, :]).T).astype(np.float32)
    wvT = ((v_w * ln1_w[None, :]).T).astype(np.float32)
    woT = o_w.T.astype(np.float32)
    wgT = ((gate_w * ln2_w[None, :]).T).astype(np.float32)
    wuT = ((up_w * ln2_w[None, :]).T).astype(np.float32)
    wdT = down_w.T.astype(np.float32)

    c = np.ascontiguousarray
    # tiled layouts (one contiguous DRAM block per SBUF partition per tile)
    wk_t = c(wkT.reshape(HT, P, NKV // 2, 256)
             .transpose(2, 1, 0, 3)).astype(bf)               # [4,128,16,256]
    wv_t = c(wvT.reshape(2, HT // 2, P, 2, 512)
             .transpose(3, 0, 2, 1, 4)).astype(bf)            # [2,2,128,8,512]
    wq_t = c(wqT.reshape(2, HT // 2, P, 4, 512)
             .transpose(3, 0, 2, 1, 4)).astype(bf)            # [4,2,128,8,512]
    wo_t = c(woT.reshape(NH, P, HT // 2, 256)
             .transpose(2, 1, 0, 3)).astype(bf)               # [8,128,16,256]
    wg_t = c(wgT.reshape(HT, P, FCP, 256)
             .transpose(2, 1, 0, 3)).astype(bf)               # [22,128,16,256]
    wu_t = c(wuT.reshape(HT, P, FCP, 256)
             .transpose(2, 1, 0, 3)).astype(bf)
    wd_t = c(wdT.reshape(4, FQ, P, HT // 2, 256)
             .transpose(3, 0, 2, 1, 4)).astype(bf)            # [8,4,128,11,256]
    return wk_t, wv_t, wq_t, wo_t, wg_t, wu_t, wd_t


def _mask_structure(m, S_):
    """Derive the global (ext, masked) structure from the [S, S] mask.

    Returns per-q-tile bank extents, {(qt, bank) -> mask slot}, and the
    column-order table mapping (bank, position) -> global key column.
    """
    NT = S_ // TPG // P
    NB = S_ // 512
    col_of = np.empty((NB, 512), np.int64)
    for bi in range(NB):
        for o in range(TPG):
            col_of[bi, o * P:(o + 1) * P] = o + TPG * (P * bi + np.arange(P))
    need = np.zeros((NT, NB), bool)
    nonzero = np.zeros((NT, NB), bool)
    for r in range(TPG):
        for qt in range(NT):
            rows = r + TPG * (P * qt + np.arange(P))
            sub = m[rows]
            for bi in range(NB):
                blk = sub[:, col_of[bi]]
                need[qt, bi] |= bool((blk > MASK_CLAMP).any())
                nonzero[qt, bi] |= bool((blk < 0).any())
    ext = []
    masked = {}
    for qt in range(NT):
        e = int(np.max(np.nonzero(need[qt])[0])) + 1 if need[qt].any() else 1
        ext.append(e)
        for bi in range(e):
            if nonzero[qt, bi]:
                masked[(qt, bi)] = len(masked)
    return ext, masked, col_of


def kernel(hidden_states, attention_mask, q_w, k_w, v_w, o_w,
           gate_w, up_w, down_w, ln1_w, ln2_w):
    hidden_states = np.asarray(hidden_states, np.float32)
    m = np.maximum(np.asarray(attention_mask, np.float32)[0, 0], MASK_CLAMP)
    S_ = hidden_states.shape[1]
    FF_ = gate_w.shape[0]
    NT = S_ // TPG // P

    ext, masked, col_of = _mask_structure(m, S_)
    n_mask = len(masked)
    nc = _get_program(S_, FF_, ext, tuple(masked.items()), n_mask)

    wk_t, wv_t, wq_t, wo_t, wg_t, wu_t, wd_t = _prep_weights(
        np.asarray(q_w, np.float32), np.asarray(k_w, np.float32),
        np.asarray(v_w, np.float32), np.asarray(o_w, np.float32),
        np.asarray(gate_w, np.float32), np.asarray(up_w, np.float32),
        np.asarray(down_w, np.float32), np.asarray(ln1_w, np.float32),
        np.asarray(ln2_w, np.float32))

    bf = ml_dtypes.bfloat16
    in_maps = []
    for core in range(NC):
        b, r = core // TPG, core % TPG
        rows = r + TPG * np.arange(S_ // TPG)
        x_own = np.ascontiguousarray(
            hidden_states[b, rows].reshape(NT, P, H))
        mask_blocks = np.zeros((max(n_mask, 1), P, 512), np.float32)
        for (qt, bi), mi in masked.items():
            qrows = r + TPG * (P * qt + np.arange(P))
            mask_blocks[mi] = m[np.ix_(qrows, col_of[bi])]
        in_maps.append({
            "x": x_own, "mask": mask_blocks.astype(bf),
            "wk": wk_t, "wv": wv_t, "wq": wq_t, "wo": wo_t,
            "wg": wg_t, "wu": wu_t, "wd": wd_t,
        })

    res = run_bass_kernel_spmd(nc, in_maps, list(range(NC)),
                               trace=bool(os.environ.get("KERNEL_TRACE")))
    global LAST_RESULT
    LAST_RESULT = res

    out = np.empty((B, S_, H), np.float32)
    for core in range(NC):
        b, r = core // TPG, core % TPG
        rows = r + TPG * np.arange(S_ // TPG)
        out[b, rows] = res.results[core]["y"].reshape(S_ // TPG, H)
    return out


# revision 12
# speedup vs baseline: 1.5005x; 1.0492x over previous
"""Trainium2 Bass kernel for a dense GQA transformer layer (pre-norm, SwiGLU MLP).

Full shapes: B=2, S=2048, H=2048, NH=16, NKV=8, HD=128, FF=5632, fp32 I/O.

Sharding across 8 NeuronCores (one SPMD program):
  core = (b, r) with b = core//4 (data-parallel over batch),
  r = core%4 (sequence-parallel, row-interleaved: core owns rows r::4 of
  batch b). Row interleaving makes the causal-attention work identical on
  every core, which a single SPMD program requires.
  K/V are computed for owned rows only and AllGather'ed (groups of 4).
  Everything else (QKV/O projections, softmax, MLP) is token-parallel with
  full weights per core. Host reassembles the row-interleaved outputs.

Perf structure (v2):
  - All weights are pre-tiled host-side so each SBUF weight tile is one
    contiguous DRAM block (8KB per partition per DMA descriptor), loaded
    with a few large dma_starts through a single ring-buffered pool.
  - V weights are reused across token tiles and Q weights across q-tiles
    (loop order oc-outer) instead of being re-DMA'ed.
  - The K/V AllGather is split in four and each piece is issued as soon
    as its producer finishes, overlapping the gather with QKV compute.
  - Softmax normalization is folded into the P-transpose as a matmul with
    diag(1/den) instead of a full-width scalar rescale pass, and the
    scores for q-tile t+1 are emitted before transpose+AV of q-tile t so
    the PE never waits on the softmax chain.

Precision: bf16 matmuls with fp32 PSUM accumulation; softmax, norms and
residuals in fp32. RMSNorm weights are folded into the following projection
weights host-side; all weights are pre-transposed host-side to [in, out].
"""

import sys

if "/opt/trn_rl_repo" not in sys.path:
    sys.path.insert(0, "/opt/trn_rl_repo")

import math
import os
import numpy as np
import ml_dtypes

import concourse.bass as bass
import concourse.bacc as bacc
import concourse.tile as tile
import concourse.mybir as mybir
from concourse.bass_utils import run_bass_kernel_spmd
from concourse.masks import make_identity

F32 = mybir.dt.float32
BF16 = mybir.dt.bfloat16
AFT = mybir.ActivationFunctionType
ALU = mybir.AluOpType

# ---- fixed problem dims ----
B, S, H = 2, 2048, 2048
NH, NKV, HD = 16, 8, 128
FF = 5632
EPS = 1e-6
NC = 8          # cores
TPG = 4         # cores per batch group (sequence-parallel ways)
P = 128         # partitions

MASK_CLAMP = -30000.0


def _build_program(S_, FF_, ext, masked, n_mask):
    """Emit the SPMD program.

    S_: sequence length, FF_: mlp width (parameterized for small-scale tests)
    ext: tuple, per q-tile number of 512-col key banks to compute
    masked: dict {(qt, bank): mask_slot_index} for banks needing a mask add
    n_mask: number of [128, 512] mask blocks in the mask input
    """
    TOWN = S_ // TPG              # tokens owned per core
    NT = TOWN // P                # q-tiles per core
    NB = S_ // 512                # key banks (512 cols each)
    HT = H // P                   # 16 H tiles
    FC = FF_ // P                 # FF tiles
    FCP = FC // 2                 # FF pair tiles (gate/up psum pairs)
    KVH = NKV
    KH = KVH // 2
    assert len(ext) == NT
    QSCALE = 1.0 / math.sqrt(HD)

    nc = bacc.Bacc("TRN2", target_bir_lowering=False, debug=False,
                   num_devices=NC)

    # ---- I/O ----
    # weights arrive pre-tiled host-side; each [128, ...] tile is one
    # contiguous DRAM block per partition.
    x_in = nc.dram_tensor("x", [NT, P, H], F32, kind="ExternalInput").ap()
    wk_t = nc.dram_tensor("wk", [KH, P, HT, 256], BF16,
                          kind="ExternalInput").ap()
    wv_t = nc.dram_tensor("wv", [2, 2, P, HT // 2, 512], BF16,
                          kind="ExternalInput").ap()
    wq_t = nc.dram_tensor("wq", [4, 2, P, HT // 2, 512], BF16,
                          kind="ExternalInput").ap()
    wo_t = nc.dram_tensor("wo", [HT // 2, P, NH, 256], BF16,
                          kind="ExternalInput").ap()
    wg_t = nc.dram_tensor("wg", [FCP, P, HT, 256], BF16,
                          kind="ExternalInput").ap()
    wu_t = nc.dram_tensor("wu", [FCP, P, HT, 256], BF16,
                          kind="ExternalInput").ap()
    wd_t = nc.dram_tensor("wd", [HT // 2, 4, P, FC // 4, 256], BF16,
                          kind="ExternalInput").ap()
    mask_in = nc.dram_tensor("mask", [max(n_mask, 1), P, 512], BF16,
                             kind="ExternalInput").ap()
    y_out = nc.dram_tensor("y", [NT, P, H], F32, kind="ExternalOutput").ap()

    # ---- internal DRAM for the K/V AllGather (split in halves for overlap) ----
    k_loc = [nc.dram_tensor(f"k_loc{i}", [KH, HD, NT, P], BF16).ap()
             for i in range(2)]
    v_loc = [nc.dram_tensor(f"v_loc{i}", [NT, P, KH, HD], BF16).ap()
             for i in range(2)]
    k_all = [nc.dram_tensor(f"k_all{i}", [TPG, KH, HD, NT, P], BF16).ap()
             for i in range(2)]
    v_all = [nc.dram_tensor(f"v_all{i}", [TPG, NT, P, KH, HD], BF16).ap()
             for i in range(2)]

    groups = [[g * TPG + i for i in range(TPG)] for g in range(NC // TPG)]

    from contextlib import ExitStack
    with ExitStack() as ctx:
        tc = ctx.enter_context(tile.TileContext(nc))
        pool = lambda name, bufs, **kw: ctx.enter_context(
            tc.tile_pool(name=name, bufs=bufs, **kw))
        singles = pool("ones", 1)
        resid_pool = pool("resid", NT)
        ybuf = pool("ybuf", HT)
        qT_pool = pool("qTp", NT)
        kv_pool = pool("kvbuf", 2)
        ybf_pool = pool("ybfp", 1)
        pbf_pool = pool("pbf", 4)
        pT_pool = pool("pTp", 3)
        aT_pool = pool("aTp", 1)
        mT_pool = pool("mTp", FC)
        mask_pool = pool("maskp", max(n_mask, 1))
        small_pool = pool("small", 8)
        w_pool = pool("wp", 4)
        cpy_pool = pool("cpy", 5)
        ptr_pool = pool("ptr", 2, space="PSUM")
        pmm_pool = pool("pmm", 5, space="PSUM")
        psc_pool = pmm_pool
        pav_pool = pool("pav", 1, space="PSUM")

        ident = singles.tile([P, P], BF16)
        make_identity(nc, ident)
        eps_c = singles.tile([P, 1], F32)
        nc.vector.memset(eps_c, EPS)

        # mask blocks (bf16: only 0 / -30000, both exact; resident).
        # Loaded on the scalar HWDGE ring to keep the sync ring free for x.
        mask_sb = []
        for mi in range(n_mask):
            mt = mask_pool.tile([P, 512], BF16, tag="mask")
            nc.scalar.dma_start(out=mt, in_=mask_in[mi])
            mask_sb.append(mt)

        def rmsnorm_to_ybf(xt):
            ssum = small_pool.tile([P, 1], F32, tag="ss")
            ybf = ybf_pool.tile([P, H], BF16, tag="ybf")
            # ybf is first used as a throwaway output of the Square pass
            # (only the accumulator matters), then overwritten in place.
            nc.scalar.activation(out=ybf, in_=xt, func=AFT.Square,
                                 accum_out=ssum)
            std = small_pool.tile([P, 1], F32, tag="std")
            nc.scalar.activation(out=std, in_=ssum, func=AFT.Sqrt,
                                 scale=1.0 / H, bias=eps_c)
            rstd = small_pool.tile([P, 1], F32, tag="rstd")
            nc.vector.reciprocal(rstd, std)
            nc.scalar.activation(out=ybf, in_=xt, func=AFT.Copy, scale=rstd)
            return ybf

        def transpose_into(dst_tiles, ybf, qt):
            # ybf [128, H] (tokens x H) -> dst_tiles[ht][:, qt*P:...] (H x tok)
            for ht in range(HT):
                ptr = ptr_pool.tile([P, 2, P], BF16, tag="tr")
                nc.tensor.transpose(ptr[:, 0, :],
                                    ybf[:, ht * P:(ht + 1) * P], ident)
                nc.vector.tensor_copy(dst_tiles[ht][:, qt * P:(qt + 1) * P],
                                      ptr[:, 0, :])

        # Y[ht] : transposed normed activations [128 (H), TOWN] bf16
        Y = [ybuf.tile([P, TOWN], BF16, tag="y", name=f"Y{i}") for i in range(HT)]
        x_tiles = []

        # ---- stage A: load x, rmsnorm1, transpose into Y ----
        # x tiles split across both HWDGE rings so the serial per-ring time
        # (~4us per 1MB tile) halves before the first rmsnorm can start.
        for qt in range(NT):
            xt = resid_pool.tile([P, H], F32, tag="x")
            x_tiles.append(xt)
            eng = nc.sync if qt % 2 == 0 else nc.scalar
            eng.dma_start(out=xt, in_=x_in[qt])
        for qt in range(NT):
            ybf = rmsnorm_to_ybf(x_tiles[qt])
            transpose_into(Y, ybf, qt)

        # ---- stage B: K/V for owned tokens, split AllGather ASAP ----
        def k_half(half):
            for kvp in (2 * half, 2 * half + 1):   # kv head pairs
                wk_sb = w_pool.tile([P, HT, 256], BF16, tag="W", name="wk_sb")
                nc.sync.dma_start(out=wk_sb, in_=wk_t[kvp])
                pk = [pmm_pool.tile([P, 512], F32, tag="mm", name=f"pk{j}")
                      for j in range(2)]
                for ht in range(HT):
                    for j in range(2):
                        nc.tensor.matmul(
                            pk[j][:, :TOWN],
                            lhsT=wk_sb[:, ht, j * P:(j + 1) * P],
                            rhs=Y[ht], start=(ht == 0), stop=(ht == HT - 1))
                for j in range(2):
                    kvh = kvp * 2 + j
                    kc = cpy_pool.tile([P, 512], BF16, tag="kc")
                    nc.vector.tensor_copy(kc[:, :TOWN], pk[j][:, :TOWN])
                    nc.sync.dma_start(
                        out=k_loc[kvh // KH][kvh % KH].rearrange(
                            "d t i -> d (t i)"),
                        in_=kc[:, :TOWN])

        def v_half(half):
            wv_sb = []
            for hh in range(2):
                wt = w_pool.tile([P, HT // 2, 512], BF16, tag="W",
                                 name="wv_sb")
                nc.sync.dma_start(out=wt, in_=wv_t[half, hh])
                wv_sb.append(wt)
            for t in range(NT):
                pv = pmm_pool.tile([P, 512], F32, tag="mm")
                for ht in range(HT):
                    nc.tensor.matmul(
                        pv, lhsT=Y[ht][:, t * P:(t + 1) * P],
                        rhs=wv_sb[ht // (HT // 2)][:, ht % (HT // 2), :],
                        start=(ht == 0), stop=(ht == HT - 1))
                vc = cpy_pool.tile([P, 512], BF16, tag="kc")
                nc.vector.tensor_copy(vc, pv)
                nc.sync.dma_start(
                    out=v_loc[half][t].rearrange("s k d -> s (k d)"),
                    in_=vc)

        def gather(i, what):
            if what == "k":
                nc.gpsimd.collective_compute(
                    "AllGather", ALU.bypass, ins=[k_loc[i].opt()],
                    outs=[k_all[i].opt()], replica_groups=groups)
            else:
                nc.gpsimd.collective_compute(
                    "AllGather", ALU.bypass, ins=[v_loc[i].opt()],
                    outs=[v_all[i].opt()], replica_groups=groups)

        k_half(0)
        gather(0, "k")
        v_half(0)
        gather(0, "v")
        k_half(1)
        gather(1, "k")
        v_half(1)
        gather(1, "v")

        # ---- stage C: Q for owned tokens -> qT[qt] [128(d), NH, 128(tq)] ----
        # oc-outer so each weight tile is loaded once and reused for all qt.
        qT = [qT_pool.tile([P, NH, P], BF16, tag="qT", name=f"qT{i}")
              for i in range(NT)]
        for oc in range(NH * HD // 512):
            wq_sb = []
            for hh in range(2):
                wt = w_pool.tile([P, HT // 2, 512], BF16, tag="W",
                                 name="wq_sb")
                nc.sync.dma_start(out=wt, in_=wq_t[oc, hh])
                wq_sb.append(wt)
            for qt in range(NT):
                pq = pmm_pool.tile([P, 512], F32, tag="mm")
                for ht in range(HT):
                    nc.tensor.matmul(
                        pq, lhsT=Y[ht][:, qt * P:(qt + 1) * P],
                        rhs=wq_sb[ht // (HT // 2)][:, ht % (HT // 2), :],
                        start=(ht == 0), stop=(ht == HT - 1))
                qc = cpy_pool.tile([P, 512], BF16, tag="kc")
                nc.vector.tensor_copy(qc, pq)
                for hh in range(4):
                    ptr = ptr_pool.tile([P, 2, P], BF16, tag="tr")
                    nc.tensor.transpose(ptr[:, 0, :],
                                        qc[:, hh * P:(hh + 1) * P], ident)
                    nc.vector.tensor_copy(qT[qt][:, oc * 4 + hh, :],
                                          ptr[:, 0, :])

        # ---- stage D: attention ----
        aT = aT_pool.tile([P, NH, TOWN], BF16, tag="aT")
        for kvh in range(KVH):
            kT_sb = kv_pool.tile([P, NB, TPG, P], BF16, tag="kT")
            v_sb = kv_pool.tile([P, NB, TPG, HD], BF16, tag="vT")
            ka, va = k_all[kvh // KH], v_all[kvh // KH]
            # SWDGE (gpsimd queue): these waits depend on the collectives,
            # and on the sync queue they head-of-line block later weight DMAs.
            for o in range(TPG):
                nc.gpsimd.dma_start(out=kT_sb[:, :, o, :], in_=ka[o, kvh % KH])
                nc.gpsimd.dma_start(
                    out=v_sb[:, :, o, :],
                    in_=va[o].rearrange("t s k d -> s t k d")[:, :, kvh % KH, :])

            def scores(qt):
                nbank = ext[qt]
                pb, dg = [], []
                for h2 in range(2):
                    h = 2 * kvh + h2
                    accs = small_pool.tile([P, NB], F32, tag="accs")
                    pbt = pbf_pool.tile([P, NB * 512], BF16, tag="pb")
                    pb.append(pbt)
                    for bi in range(nbank):
                        ps = psc_pool.tile([P, 512], F32, tag="mm")
                        nc.tensor.matmul(
                            ps, lhsT=qT[qt][:, h, :],
                            rhs=kT_sb[:, bi, :, :].rearrange(
                                "d o i -> d (o i)"),
                            start=True, stop=True)
                        mi = masked.get((qt, bi))
                        if mi is not None:
                            nc.vector.tensor_add(ps, ps, mask_sb[mi])
                        nc.scalar.activation(
                            out=pbt[:, bi * 512:(bi + 1) * 512], in_=ps,
                            func=AFT.Exp, accum_out=accs[:, bi:bi + 1])
                    den = small_pool.tile([P, 1], F32, tag="den")
                    nc.vector.tensor_reduce(den, accs[:, :nbank],
                                            mybir.AxisListType.X, ALU.add)
                    rec = small_pool.tile([P, 1], F32, tag="rec")
                    nc.vector.reciprocal(rec, den)
                    # diag(1/den): folded into the P-transpose matmul
                    dgt = small_pool.tile([P, P], BF16, tag="diag")
                    nc.vector.tensor_scalar_mul(dgt, ident, rec)
                    dg.append(dgt)
                return pb, dg

            def trans_av(qt, pb, dg):
                # P-transposes run one chunk-pair ahead of the AV accumulation
                # (so the in-order PE never waits on the psum->sbuf copy), and
                # the copies are batched two chunks at a time to halve DVE
                # per-instruction overhead.
                nbank = ext[qt]
                pav = pav_pool.tile([P, 2, P], F32, tag="av")
                npc = nbank * TPG          # always even (TPG=4)
                pT_sbs = []

                def av(pc):
                    nc.tensor.matmul(
                        pav, lhsT=v_sb[:, pc // TPG, pc % TPG, :],
                        rhs=pT_sbs[pc // 2][:, pc % 2].rearrange(
                            "s h i -> s (h i)"),
                        start=(pc == 0), stop=(pc == npc - 1))

                for pg in range(npc // 2):
                    pT_ps = ptr_pool.tile([P, 2, 2, P], F32, tag="tr")
                    for pj in range(2):
                        pc = 2 * pg + pj
                        for h2 in range(2):
                            nc.tensor.matmul(
                                pT_ps[:, pj, h2, :],
                                lhsT=pb[h2][:, pc * P:(pc + 1) * P],
                                rhs=dg[h2], start=True, stop=True)
                    pT_sb = pT_pool.tile([P, 2, 2, P], BF16, tag="pT")
                    nc.vector.tensor_copy(pT_sb, pT_ps)
                    pT_sbs.append(pT_sb)
                    if pg >= 1:
                        av(2 * pg - 2)
                        av(2 * pg - 1)
                av(npc - 2)
                av(npc - 1)
                nc.vector.tensor_copy(
                    aT[:, 2 * kvh:2 * kvh + 2, qt * P:(qt + 1) * P], pav)

            prev = None
            for qt in range(NT):
                cur = (qt, *scores(qt))
                if prev is not None:
                    trans_av(*prev)
                prev = cur
            trans_av(*prev)

        # ---- stage E: O projection, streamed transpose + residual into x ----
        for hcp in range(HT // 2):         # H-column pairs
            wo_sb = w_pool.tile([P, NH, 256], BF16, tag="W", name="wo_sb")
            nc.sync.dma_start(out=wo_sb, in_=wo_t[hcp])
            po = [pmm_pool.tile([P, 512], F32, tag="mm", name=f"po{j}")
                  for j in range(2)]
            for h in range(NH):
                for j in range(2):
                    nc.tensor.matmul(po[j][:, :TOWN],
                                     lhsT=wo_sb[:, h, j * P:(j + 1) * P],
                                     rhs=aT[:, h, :], start=(h == 0),
                                     stop=(h == NH - 1))
            for j in range(2):
                hc = hcp * 2 + j
                oc_ = cpy_pool.tile([P, 512], BF16, tag="kc")
                nc.vector.tensor_copy(oc_[:, :TOWN], po[j][:, :TOWN])
                for qt in range(NT):
                    ptr = ptr_pool.tile([P, 2, P], BF16, tag="tr")
                    nc.tensor.transpose(ptr[:, 0, :],
                                        oc_[:, qt * P:(qt + 1) * P], ident)
                    xb = x_tiles[qt][:, hc * P:(hc + 1) * P]
                    nc.vector.tensor_add(xb, xb, ptr[:, 0, :])

        # ---- rmsnorm2 -> Y2 ----
        Y2 = [ybuf.tile([P, TOWN], BF16, tag="y", name=f"Y2_{i}")
              for i in range(HT)]
        for qt in range(NT):
            ybf = rmsnorm_to_ybf(x_tiles[qt])
            transpose_into(Y2, ybf, qt)

        # ---- stage F: MLP ----
        mT = []
        for fcp in range(FCP):             # FF-tile pairs
            wg_sb = w_pool.tile([P, HT, 256], BF16, tag="W", name="wg_sb")
            nc.sync.dma_start(out=wg_sb, in_=wg_t[fcp])
            wu_sb = w_pool.tile([P, HT, 256], BF16, tag="W", name="wu_sb")
            nc.sync.dma_start(out=wu_sb, in_=wu_t[fcp])
            pg = [pmm_pool.tile([P, 512], F32, tag="mm", name=f"pg{j}")
                  for j in range(2)]
            for ht in range(HT):
                for j in range(2):
                    nc.tensor.matmul(pg[j][:, :TOWN],
                                     lhsT=wg_sb[:, ht, j * P:(j + 1) * P],
                                     rhs=Y2[ht], start=(ht == 0),
                                     stop=(ht == HT - 1))
            pu = [psc_pool.tile([P, 512], F32, tag="mm", name=f"pu{j}")
                  for j in range(2)]
            for ht in range(HT):
                for j in range(2):
                    nc.tensor.matmul(pu[j][:, :TOWN],
                                     lhsT=wu_sb[:, ht, j * P:(j + 1) * P],
                                     rhs=Y2[ht], start=(ht == 0),
                                     stop=(ht == HT - 1))
            for j in range(2):
                sg = cpy_pool.tile([P, 512], BF16, tag="kc")
                nc.scalar.activation(out=sg[:, :TOWN], in_=pg[j][:, :TOWN],
                                     func=AFT.Silu)
                mt = mT_pool.tile([P, TOWN], BF16, tag="mT")
                mT.append(mt)
                nc.vector.tensor_mul(mt, sg[:, :TOWN], pu[j][:, :TOWN])

        FQ = FC // 4
        for hcp in range(HT // 2):
            wd_sb = []
            for q in range(4):
                wt = w_pool.tile([P, FQ, 256], BF16, tag="W", name="wd_sb")
                nc.sync.dma_start(out=wt, in_=wd_t[hcp, q])
                wd_sb.append(wt)
            pd = [pmm_pool.tile([P, 512], F32, tag="mm", name=f"pd{j}")
                  for j in range(2)]
            for fc in range(FC):
                for j in range(2):
                    nc.tensor.matmul(
                        pd[j][:, :TOWN],
                        lhsT=wd_sb[fc // FQ][:, fc % FQ, j * P:(j + 1) * P],
                        rhs=mT[fc], start=(fc == 0), stop=(fc == FC - 1))
            for j in range(2):
                hc = hcp * 2 + j
                dc = cpy_pool.tile([P, 512], BF16, tag="kc")
                nc.vector.tensor_copy(dc[:, :TOWN], pd[j][:, :TOWN])
                for qt in range(NT):
                    ptr = ptr_pool.tile([P, 2, P], BF16, tag="tr")
                    nc.tensor.transpose(ptr[:, 0, :],
                                        dc[:, qt * P:(qt + 1) * P], ident)
                    xb = x_tiles[qt][:, hc * P:(hc + 1) * P]
                    nc.vector.tensor_add(xb, xb, ptr[:, 0, :])

        for qt in range(NT):
            nc.sync.dma_start(out=y_out[qt], in_=x_tiles[qt])

    nc.compile()
    return nc


_CACHE = {}
LAST_RESULT = None


def _get_program(S_, FF_, ext, masked_items, n_mask):
    key = (S_, FF_, tuple(ext), tuple(sorted(masked_items)), n_mask)
    if key not in _CACHE:
        _CACHE[key] = _build_program(S_, FF_, tuple(ext), dict(masked_items),
                                     n_mask)
    return _CACHE[key]


def _prep_weights(q_w, k_w, v_w, o_w, gate_w, up_w, down_w, ln1_w, ln2_w):
    bf = ml_dtypes.bfloat16
    HT = H // P
    FF_ = gate_w.shape[0]
    FC = FF_ // P
    FCP = FC // 2
    FQ = FC // 4
    wqT = ((q_w * ln1_w[None, :]).T * (1.0 / math.sqrt(HD))).astype(np.float32)
    wkT = ((k_w * ln1_w[None, :]).T).astype(np.float32)
    wvT = ((v_w * ln1_w[None, :]).T).astype(np.float32)
    woT = o_w.T.astype(np.float32)
    wgT = ((gate_w * ln2_w[None, :]).T).astype(np.float32)
    wuT = ((up_w * ln2_w[None, :]).T).astype(np.float32)
    wdT = down_w.T.astype(np.float32)

    c = np.ascontiguousarray
    # tiled layouts (one contiguous DRAM block per SBUF partition per tile)
    wk_t = c(wkT.reshape(HT, P, NKV // 2, 256)
             .transpose(2, 1, 0, 3)).astype(bf)               # [4,128,16,256]
    wv_t = c(wvT.reshape(2, HT // 2, P, 2, 512)
             .transpose(3, 0, 2, 1, 4)).astype(bf)            # [2,2,128,8,512]
    wq_t = c(wqT.reshape(2, HT // 2, P, 4, 512)
             .transpose(3, 0, 2, 1, 4)).astype(bf)            # [4,2,128,8,512]
    wo_t = c(woT.reshape(NH, P, HT // 2, 256)
             .transpose(2, 1, 0, 3)).astype(bf)               # [8,128,16,256]
    wg_t = c(wgT.reshape(HT, P, FCP, 256)
             .transpose(2, 1, 0, 3)).astype(bf)               # [22,128,16,256]
    wu_t = c(wuT.reshape(HT, P, FCP, 256)
             .transpose(2, 1, 0, 3)).astype(bf)
    wd_t = c(wdT.reshape(4, FQ, P, HT // 2, 256)
             .transpose(3, 0, 2, 1, 4)).astype(bf)            # [8,4,128,11,256]
    return wk_t, wv_t, wq_t, wo_t, wg_t, wu_t, wd_t


def _mask_structure(m, S_):
    """Derive the global (ext, masked) structure from the [S, S] mask.

    Returns per-q-tile bank extents, {(qt, bank) -> mask slot}, and the
    column-order table mapping (bank, position) -> global key column.
    """
    NT = S_ // TPG // P
    NB = S_ // 512
    col_of = np.empty((NB, 512), np.int64)
    for bi in range(NB):
        for o in range(TPG):
            col_of[bi, o * P:(o + 1) * P] = o + TPG * (P * bi + np.arange(P))
    need = np.zeros((NT, NB), bool)
    nonzero = np.zeros((NT, NB), bool)
    for r in range(TPG):
        for qt in range(NT):
            rows = r + TPG * (P * qt + np.arange(P))
            sub = m[rows]
            for bi in range(NB):
                blk = sub[:, col_of[bi]]
                need[qt, bi] |= bool((blk > MASK_CLAMP).any())
                nonzero[qt, bi] |= bool((blk < 0).any())
    ext = []
    masked = {}
    for qt in range(NT):
        e = int(np.max(np.nonzero(need[qt])[0])) + 1 if need[qt].any() else 1
        ext.append(e)
        for bi in range(e):
            if nonzero[qt, bi]:
                masked[(qt, bi)] = len(masked)
    return ext, masked, col_of


def kernel(hidden_states, attention_mask, q_w, k_w, v_w, o_w,
           gate_w, up_w, down_w, ln1_w, ln2_w):
    hidden_states = np.asarray(hidden_states, np.float32)
    m = np.maximum(np.asarray(attention_mask, np.float32)[0, 0], MASK_CLAMP)
    S_ = hidden_states.shape[1]
    FF_ = gate_w.shape[0]
    NT = S_ // TPG // P

    ext, masked, col_of = _mask_structure(m, S_)
    n_mask = len(masked)
    nc = _get_program(S_, FF_, ext, tuple(masked.items()), n_mask)

    wk_t, wv_t, wq_t, wo_t, wg_t, wu_t, wd_t = _prep_weights(
        np.asarray(q_w, np.float32), np.asarray(k_w, np.float32),
        np.asarray(v_w, np.float32), np.asarray(o_w, np.float32),
        np.asarray(gate_w, np.float32), np.asarray(up_w, np.float32),
        np.asarray(down_w, np.float32), np.asarray(ln1_w, np.float32),
        np.asarray(ln2_w, np.float32))

    bf = ml_dtypes.bfloat16
    in_maps = []
    for core in range(NC):
        b, r = core // TPG, core % TPG
        rows = r + TPG * np.arange(S_ // TPG)
        x_own = np.ascontiguousarray(
            hidden_states[b, rows].reshape(NT, P, H))
        mask_blocks = np.zeros((max(n_mask, 1), P, 512), np.float32)
        for (qt, bi), mi in masked.items():
            qrows = r + TPG * (P * qt + np.arange(P))
            mask_blocks[mi] = m[np.ix_(qrows, col_of[bi])]
        in_maps.append({
            "x": x_own, "mask": mask_blocks.astype(bf),
            "wk": wk_t, "wv": wv_t, "wq": wq_t, "wo": wo_t,
            "wg": wg_t, "wu": wu_t, "wd": wd_t,
        })

    res = run_bass_kernel_spmd(nc, in_maps, list(range(NC)),
                               trace=bool(os.environ.get("KERNEL_TRACE")))
    global LAST_RESULT
    LAST_RESULT = res

    out = np.empty((B, S_, H), np.float32)
    for core in range(NC):
        b, r = core // TPG, core % TPG
        rows = r + TPG * np.arange(S_ // TPG)
        out[b, rows] = res.results[core]["y"].reshape(S_ // TPG, H)
    return out
